# revision 1
# baseline (speedup 1.0000x reference)
"""BertBiLSTMCRF loss kernel for 8 Trainium2 NeuronCores.

Sharding: data-parallel over batch (B=32 -> 4 sentences/core). The BERT
encoder (>95% of FLOPs) runs on-device in raw Bass. Activations are kept
in transposed layout hT=[H, tokens] on chip so every GEMM consumes
weights in their stored [in,out] layout as lhsT with no activation
transposes; attention computes S^T (k on partitions), uses unnormalized
exp (scores are tiny after LN + 0.02-scale weights) and gets the softmax
denominator via a ones-column matmul, so no partition-dim max/sum is
ever needed. The BiLSTM/CRF tail (small FLOPs, serial scans) runs on
host, as does the embedding gather.
"""
import os
import numpy as np
from scipy.special import erf

V, H, NL, NH, S, B, HL, T = 30522, 768, 12, 12, 256, 32, 256, 9
DH = H // NH
FF = 4 * H
NCORES = 8
BL = B // NCORES          # sentences per core
TOK = BL * S              # tokens per core (1024)
KT = H // 128             # 6 k-tiles over hidden
MT_TOK = TOK // 128       # 8 token m-tiles
FP32R = os.environ.get("KERNEL_NO_FP32R", "") == ""
USE_DEVICE = os.environ.get("KERNEL_HOST", "") == ""
DEV_LAYERS = int(os.environ.get("KERNEL_LAYERS", str(NL)))

LAST_HW_NS = None
_CACHE = {}


# ---------------------------------------------------------------- host math
def _ln_np(x, g, b):
    m = x.mean(-1, keepdims=True)
    v = ((x - m) ** 2).mean(-1, keepdims=True)
    return (x - m) / np.sqrt(v + 1e-12) * g + b


def _gelu_np(x):
    return (0.5 * x * (1.0 + erf(x / np.float32(np.sqrt(2.0))))).astype(np.float32)


def _sigmoid_np(x):
    return 1.0 / (1.0 + np.exp(-x))


def _bert_host(h, a, n_layers=NL):
    Bc = h.shape[0]
    for l in range(n_layers):
        qkv = h @ a['Wqkv'][l] + a['bqkv'][l]
        q, k, v = [t.reshape(Bc, S, NH, DH) for t in np.split(qkv, 3, axis=-1)]
        sc = np.einsum('bqhd,bkhd->bhqk', q, k) / np.float32(np.sqrt(DH))
        sc = sc - sc.max(-1, keepdims=True)
        p = np.exp(sc)
        p = p / p.sum(-1, keepdims=True)
        ctx = np.einsum('bhqk,bkhd->bqhd', p, v).reshape(Bc, S, H)
        h = _ln_np(h + ctx @ a['Wo'][l] + a['bo'][l], a['ln1_g'][l], a['ln1_b'][l])
        ff = _gelu_np(h @ a['W1'][l] + a['b1'][l]) @ a['W2'][l] + a['b2'][l]
        h = _ln_np(h + ff, a['ln2_g'][l], a['ln2_b'][l])
    return h


def _lstm_host(x, Wih, Whh, bih, bhh, reverse):
    Bc = x.shape[0]
    pre = np.swapaxes(x, 0, 1) @ Wih.T + (bih + bhh)  # [S,B,4H]
    hs = np.zeros((S, Bc, HL), np.float32)
    h = np.zeros((Bc, HL), np.float32)
    c = np.zeros((Bc, HL), np.float32)
    order = range(S - 1, -1, -1) if reverse else range(S)
    for t in order:
        g = pre[t] + h @ Whh.T
        i, f, gg, o = np.split(g, 4, axis=-1)
        c = _sigmoid_np(f) * c + _sigmoid_np(i) * np.tanh(gg)
        h = _sigmoid_np(o) * np.tanh(c)
        hs[t] = h
    return np.swapaxes(hs, 0, 1)


def _logsumexp(a, axis):
    m = a.max(axis=axis, keepdims=True)
    return (np.log(np.exp(a - m).sum(axis=axis, keepdims=True)) + m).squeeze(axis)


def _crf_host(logits, labels, maskf, crf_start, crf_end, crf_trans):
    em = np.take_along_axis(logits, labels[..., None], -1)[..., 0]
    tr = crf_trans[labels[:, :-1], labels[:, 1:]]
    last_idx = maskf.sum(1).astype(np.int32) - 1
    last_tag = np.take_along_axis(labels, last_idx[:, None], 1)[:, 0]
    num = (crf_start[labels[:, 0]] + em[:, 0]
           + ((em[:, 1:] + tr) * maskf[:, 1:]).sum(1) + crf_end[last_tag])
    alpha = crf_start + logits[:, 0]
    for t in range(1, S):
        nxt = _logsumexp(alpha[:, :, None] + crf_trans[None] + logits[:, t][:, None, :], 1)
        alpha = np.where(maskf[:, t][:, None] > 0, nxt, alpha)
    den = _logsumexp(alpha + crf_end, -1)
    return den - num


# ------------------------------------------------------------ device program
class Prog:
    """Raw-Bass multi-engine program recorder with conservative sync:
    each op waits until everything its producer engines emitted so far is
    done. Duplicate waits are elided per consumer engine. The DMA
    semaphore rotates per layer to stay far from counter limits."""

    def __init__(self):
        self.ops = {e: [] for e in ("pe", "act", "dve", "dma")}
        self.counts = {}              # sem name -> emitted count
        self.seen = {e: {} for e in self.ops}
        self.cur_dma = "dmaS0"
        self.sem_names = {"pe", "act", "dve", "dmaS0"}

    def next_dma_sem(self, name):
        self.cur_dma = name
        self.sem_names.add(name)

    def _resolve(self, dep):
        if dep == "dma":
            return [s for s in self.sem_names if s.startswith("dmaS")]
        return [dep]

    def emit(self, engine, fn, deps=()):
        waits = []
        for d in deps:
            for sem in self._resolve(d):
                if sem == engine:
                    continue
                val = self.counts.get(sem, 0)
                if val > 0 and self.seen[engine].get(sem, -1) < val:
                    waits.append((sem, val))
                    self.seen[engine][sem] = val
        sem_self = self.cur_dma if engine == "dma" else engine
        inc = 16 if engine == "dma" else 1
        self.counts[sem_self] = self.counts.get(sem_self, 0) + inc
        self.ops[engine].append((waits, fn, sem_self, inc))

    def replay(self, engine, eng, sems):
        for waits, fn, sem_self, inc in self.ops[engine]:
            for name, val in waits:
                eng.wait_ge(sems[name], val)
            fn().then_inc(sems[sem_self], inc)


def _build_encoder(n_layers):
    import concourse.bass as bass
    import concourse.mybir as mybir
    from contextlib import ExitStack
    dt = mybir.dt
    f32 = dt.float32
    AF = mybir.ActivationFunctionType
    ALU = mybir.AluOpType

    nc = bass.Bass()
    ctx = ExitStack()

    def mmdt(ap):
        return ap.bitcast(dt.float32r) if FP32R else ap

    def R(ap):
        # round-on-write for tiles later consumed by fp32r matmuls
        return ap.bitcast(dt.float32r) if FP32R else ap

    # ---- DRAM parameters
    hT0 = nc.declare_dram_parameter("hT0", [H, TOK], f32, isOutput=False)
    Wqkv = nc.declare_dram_parameter("Wqkv", [NL, H, 3 * H], f32, isOutput=False)
    Wo = nc.declare_dram_parameter("Wo", [NL, H, H], f32, isOutput=False)
    W1 = nc.declare_dram_parameter("W1", [NL, H, FF], f32, isOutput=False)
    W2 = nc.declare_dram_parameter("W2", [NL, FF, H], f32, isOutput=False)
    biasall = nc.declare_dram_parameter("biasall", [NL, 128, 80], f32, isOutput=False)
    consts = nc.declare_dram_parameter("consts", [128, 1024], f32, isOutput=False)
    onesd = nc.declare_dram_parameter("onesd", [128, 1], f32, isOutput=False)
    hTout = nc.declare_dram_parameter("hTout", [H, TOK], f32, isOutput=True)
    zscr = nc.dram_tensor("zscr", [4, 3072], f32)

    # ---- on-chip tensors
    sbt = lambda nm, shape: ctx.enter_context(nc.sbuf_tensor(nm, shape, f32))
    hT = sbt("hT", [128, KT, TOK])
    h1T = sbt("h1T", [128, KT, TOK])
    ctxT = sbt("ctxT", [128, KT * TOK])   # flat; viewed [128, 6, 1024]
    big = sbt("bigb", [128, 12, TOK])     # qkT in attn; ff1 tiles 0-11; LN sq
    vbuf = sbt("vbuf", [128, KT * TOK])   # flat; v=[128,8,768] / ff1 18-23
    wsl = sbt("wsl", [128, 2, 3072])      # weight slab, 2 slots
    bias = sbt("biassb", [128, 80])
    csts = sbt("csts", [128, 904])
    stats = sbt("stats", [1, 2048])   # col blocks: mean | E2/var/istd
    stats2 = sbt("stats2", [1, 1024])  # istd
    zbuf4 = sbt("zbuf4", [97, 3072])   # Z at partition bases 0/32/64/96
    zbuf = sbt("zbuf", [12, 1024])     # Z reshaped for broadcast matmul
    expS = sbt("expS", [128, 2, S])
    onesr = sbt("onesr", [128, 1])

    psA = ctx.enter_context(nc.psum_tensor("psA", [128, 1024], f32))
    psB = ctx.enter_context(nc.psum_tensor("psB", [128, 1024], f32))
    psS = ctx.enter_context(nc.psum_tensor("psS", [128, 2, S], f32))
    psC = ctx.enter_context(nc.psum_tensor("psC", [128, S], f32))

    ctxTv = ctxT[:, :].rearrange("p (n t) -> p n t", t=TOK)

    def vtile(m):                     # v token-tile m: [128, 768]
        return vbuf[:, m * H:(m + 1) * H]

    def fftile(kt):                   # ff1 feature k-tile: [128, 1024]
        if kt < 12:
            return big[:, kt, :]
        if kt < 18:
            return ctxTv[:, kt - 12, :]
        return vbuf[:, (kt - 18) * TOK:(kt - 17) * TOK]

    P = Prog()
    CD = ("pe", "act", "dve", "dma")

    def dma(dst, src, deps=("pe", "act", "dve")):
        P.emit("dma", lambda d=dst, s=src: nc.sync.dma_start(out=d, in_=s),
               deps=deps)

    def mm(out, lhsT, rhs, start, stop, raw=False):
        if raw:
            P.emit("pe", lambda o=out, l=lhsT, r=rhs, a=start, b=stop:
                   nc.tensor.matmul(o, l, r, start=a, stop=b), deps=CD)
        else:
            P.emit("pe", lambda o=out, l=lhsT, r=rhs, a=start, b=stop:
                   nc.tensor.matmul(o, mmdt(l), mmdt(r), start=a, stop=b),
                   deps=CD)

    def act(out, in_, func, b=0.0, scale=1.0):
        P.emit("act", lambda o=out, i=in_, f=func, bb=b, s=scale:
               nc.scalar.activation(o, i, f, bias=bb, scale=s), deps=CD)

    def dve_tt(out, in0, in1, op):
        P.emit("dve", lambda o=out, x=in0, y=in1, z=op:
               nc.vector.tensor_tensor(o, x, y, z), deps=CD)

    def dve_ts(out, in_, s1, s2, op0, op1):
        P.emit("dve", lambda o=out, i=in_, a=s1, b=s2, x=op0, y=op1:
               nc.vector.tensor_scalar(o, i, a, b, x, y), deps=CD)

    def dve_recip(out, in_):
        P.emit("dve", lambda o=out, i=in_: nc.vector.reciprocal(o, i), deps=CD)

    # ---- boot: constants + initial activations
    dma(csts[:, :], consts[:, 0:904], deps=())
    dma(R(onesr[:, :]), R(onesd[:, :]), deps=())
    dma(R(hT[:, :, :]), R(hT0.rearrange("(n p) t -> p n t", p=128)), deps=())
    ones128 = onesr[:, 0:1]
    onesrow = csts[0:1, 2:130]        # [1,128] ones on partition 0

    def m12(m):                       # [12, 128] head-broadcast map k-tile
        return csts[0:12, 130 + m * 128:130 + (m + 1) * 128]

    def stream_gemm(W_dram, n_in, n_out, rhs_tile_fn, out_fn, bias_fn,
                    act_fn):
        """out[m] = act(sum_kt W[kt,m].T @ rhs[kt] + bias[m]); W streamed
        through wsl slots (per m-tile)."""
        kt_n = n_in // 128
        mt_n = n_out // 128
        for m in range(mt_n):
            slot = wsl[:, m % 2, :]
            for kt in range(kt_n):
                dma(R(slot[:, kt * 128:(kt + 1) * 128]),
                    R(W_dram[kt * 128:(kt + 1) * 128, m * 128:(m + 1) * 128]),
                    deps=("pe",))
            for half in range(2):
                ps = psA[:, half * 512:(half + 1) * 512]
                for kt in range(kt_n):
                    mm(ps, slot[:, kt * 128:(kt + 1) * 128],
                       rhs_tile_fn(kt)[:, half * 512:(half + 1) * 512],
                       start=(kt == 0), stop=(kt == kt_n - 1))
            act(R(out_fn(m)), psA[:, :TOK], act_fn, b=bias_fn(m))

    def layernorm(x, gcol0, bcol0, gbuf, sq):
        # x: [128, KT, TOK] feature-major; returns in place
        for kt in range(KT):
            act(R(sq[:, kt, :]), x[:, kt, :], AF.Square)
        for half in range(2):
            c0, c1 = half * 512, (half + 1) * 512
            for kt in range(KT):
                mm(psA[0:1, c0:c1], ones128, x[:, kt, c0:c1],
                   start=(kt == 0), stop=(kt == KT - 1))
            for kt in range(KT):
                mm(psB[0:1, c0:c1], ones128, sq[:, kt, c0:c1],
                   start=(kt == 0), stop=(kt == KT - 1))
        mean = stats[0:1, 0:1024]
        blk = stats[0:1, 1024:2048]       # E2 -> var -> istd, in place
        tmp = stats2[0:1, :]              # meansq -> sd
        act(mean, psA[0:1, :], AF.Identity, scale=1.0 / H)
        act(blk, psB[0:1, :], AF.Identity, scale=1.0 / H)
        dve_tt(tmp, mean, mean, ALU.mult)
        dve_tt(blk, blk, tmp, ALU.subtract)
        P.emit("dve", lambda: nc.vector.tensor_scalar_add(blk, blk, 1e-12),
               deps=CD)
        act(tmp, blk, AF.Sqrt)
        dve_recip(blk, tmp)                                           # istd
        for half in range(2):
            c0, c1 = half * 512, (half + 1) * 512
            mm(psA[:, c0:c1], onesrow, stats[0:1, c0:c1], start=True,
               stop=True, raw=True)
            mm(psB[:, c0:c1], onesrow, stats[0:1, 1024 + c0:1024 + c1],
               start=True, stop=True, raw=True)
        for kt in range(KT):
            dve_tt(R(x[:, kt, :]), x[:, kt, :], psA[:, :TOK], ALU.subtract)
            dve_tt(R(x[:, kt, :]), x[:, kt, :], psB[:, :TOK], ALU.mult)
            dve_ts(R(x[:, kt, :]), x[:, kt, :],
                   gbuf[:, gcol0 + kt:gcol0 + kt + 1],
                   gbuf[:, bcol0 + kt:bcol0 + kt + 1], ALU.mult, ALU.add)

    for l in range(n_layers):
        P.next_dma_sem(f"dmaS{l + 1}")
        dma(bias[:, :], biasall[l])

        # qkT into big[:, 0:12]: features q(0-5) k(6-11)
        stream_gemm(Wqkv[l][:, 0:1536], H, 1536, lambda kt: hT[:, kt, :],
                    lambda m: big[:, m, :], lambda m: bias[:, m:m + 1],
                    AF.Identity)

        # v = hT.T @ Wv  (token-major; bias folded in after softmax)
        for kt in range(KT):
            dma(R(wsl[:, kt % 2, (kt // 2) * 768:(kt // 2) * 768 + 768]),
                R(Wqkv[l][kt * 128:(kt + 1) * 128, 1536:2304]), deps=("pe",))
        for m in range(MT_TOK):
            for c0, c1 in ((0, 512), (512, 768)):
                ps = psA[:, c0:c1]
                for kt in range(KT):
                    wv = wsl[:, kt % 2, (kt // 2) * 768:(kt // 2) * 768 + 768]
                    mm(ps, hT[:, kt, m * 128:(m + 1) * 128], wv[:, c0:c1],
                       start=(kt == 0), stop=(kt == KT - 1))
            act(R(vtile(m)), psA[:, 0:H], AF.Identity)

        # attention
        for s in range(BL):
            for hh in range(NH):
                prow = 64 * (hh % 2)
                qt = big[prow:prow + 64, hh // 2, s * S:(s + 1) * S]
                ktap = big[prow:prow + 64, 6 + hh // 2, s * S:(s + 1) * S]
                for i in range(2):
                    mm(psS[:, i, :], ktap[:, i * 128:(i + 1) * 128], qt,
                       start=True, stop=True)
                act(R(expS[:, :, :]), psS[:, :, :], AF.Exp, scale=1.0 / 8.0)
                for i in range(2):
                    mm(psC[0:64, :], vtile(2 * s + i)[:, hh * 64:(hh + 1) * 64],
                       expS[:, i, :], start=(i == 0), stop=(i == 1))
                    mm(psS[0:1, 0, :], ones128, expS[:, i, :],
                       start=(i == 0), stop=(i == 1))
                act(R(ctxTv[prow:prow + 64, hh // 2, s * S:(s + 1) * S]),
                    psC[0:64, :], AF.Identity)
                zr = zbuf4[32 * (hh % 4):32 * (hh % 4) + 1,
                           (hh // 4) * 1024 + s * S:(hh // 4) * 1024 + (s + 1) * S]
                act(zr, psS[0:1, 0, :], AF.Identity)

        # normalize ctx by Z (per head), add v bias
        for p4 in range(4):
            dve_recip(zbuf4[32 * p4:32 * p4 + 1, :], zbuf4[32 * p4:32 * p4 + 1, :])
        dma(zscr[:, :], zbuf4[0:97:32, :])
        dma(zbuf[0:12, :], zscr[:, :].rearrange("p (b t) -> (p b) t", b=3))
        for m in range(KT):
            for half in range(2):
                mm(psA[:, half * 512:(half + 1) * 512], m12(m),
                   zbuf[0:12, half * 512:(half + 1) * 512], start=True,
                   stop=True, raw=True)
            dve_tt(R(ctxTv[:, m, :]), ctxTv[:, m, :], psA[:, :TOK], ALU.mult)
            P.emit("dve", lambda m=m: nc.vector.tensor_scalar_add(
                R(ctxTv[:, m, :]), ctxTv[:, m, :], bias[:, 12 + m:13 + m]),
                deps=CD)

        # attn proj + residual + LN1
        stream_gemm(Wo[l], H, H, lambda kt: ctxTv[:, kt, :],
                    lambda m: h1T[:, m, :], lambda m: bias[:, 18 + m:19 + m],
                    AF.Identity)
        for m in range(KT):
            dve_tt(R(h1T[:, m, :]), h1T[:, m, :], hT[:, m, :], ALU.add)
        layernorm(h1T, 24, 30, bias, big[:, 0:KT, :])

        # FF1 (gelu) into big/ctxT/vbuf tiles
        stream_gemm(W1[l], H, FF, lambda kt: h1T[:, kt, :],
                    fftile, lambda m: bias[:, 36 + m:37 + m], AF.Gelu)

        # FF2 + residual + LN2 -> hT
        for m in range(KT):
            slot = wsl[:, m % 2, :]
            for kt in range(24):
                dma(R(slot[:, kt * 128:(kt + 1) * 128]),
                    R(W2[l][kt * 128:(kt + 1) * 128, m * 128:(m + 1) * 128]),
                    deps=("pe",))
            for half in range(2):
                ps = psA[:, half * 512:(half + 1) * 512]
                for kt in range(24):
                    mm(ps, slot[:, kt * 128:(kt + 1) * 128],
                       fftile(kt)[:, half * 512:(half + 1) * 512],
                       start=(kt == 0), stop=(kt == 23))
            act(R(hT[:, m, :]), psA[:, :TOK], AF.Identity, b=bias[:, 60 + m:61 + m])
            dve_tt(R(hT[:, m, :]), hT[:, m, :], h1T[:, m, :], ALU.add)
        layernorm(hT, 66, 72, bias, big[:, 0:KT, :])

    dma(hTout.rearrange("(n p) t -> p n t", p=128), hT[:, :, :])

    # ---- replay into engine blocks
    sems = {}
    for name in sorted(P.sem_names):
        sems[name] = ctx.enter_context(nc.semaphore(name))
    with nc.Block() as block:
        @block.tensor
        def _(eng):
            P.replay("pe", eng, sems)

        @block.scalar
        def _(eng):
            P.replay("act", eng, sems)

        @block.vector
        def _(eng):
            P.replay("dve", eng, sems)

        @block.sync
        def _(eng):
            P.replay("dma", eng, sems)

    return nc, ctx


def _pack_consts():
    c = np.zeros((128, 1024), np.float32)
    c[:, 0] = 1.0                       # ones128
    c[0, 2:130] = 1.0                   # onesrow
    # zbuf row r (after the strided reshape DMA) holds head (r%3)*4 + r//3
    for r in range(NH):
        hh = (r % 3) * 4 + r // 3
        for f in range(H):
            if f // DH == hh:
                c[r, 130 + f] = 1.0
    return c


def _pack_bias(a):
    out = np.zeros((NL, 128, 80), np.float32)

    def col(vec):                       # feature vec [n*128] -> [128, n]
        return vec.reshape(-1, 128).T

    for l in range(NL):
        out[l, :, 0:18] = col(a['bqkv'][l])
        out[l, :, 18:24] = col(a['bo'][l])
        out[l, :, 24:30] = col(a['ln1_g'][l])
        out[l, :, 30:36] = col(a['ln1_b'][l])
        out[l, :, 36:60] = col(a['b1'][l])
        out[l, :, 60:66] = col(a['b2'][l])
        out[l, :, 66:72] = col(a['ln2_g'][l])
        out[l, :, 72:78] = col(a['ln2_b'][l])
    return out


def run_device(h0, a):
    global LAST_HW_NS
    if not USE_DEVICE:
        return _bert_host(h0, a)
    import time
    from concourse.bass_utils import run_bass_kernel_spmd

    key = ("enc", DEV_LAYERS)
    if key not in _CACHE:
        _CACHE[key] = _build_encoder(DEV_LAYERS)
    nc, _ctx = _CACHE[key]

    biasall = _pack_bias(a)
    consts = _pack_consts()
    shared = {"Wqkv": a['Wqkv'], "Wo": a['Wo'], "W1": a['W1'], "W2": a['W2'],
              "biasall": biasall, "consts": consts,
              "onesd": np.ones((128, 1), np.float32)}
    in_maps = []
    for c in range(NCORES):
        hc = h0[c * BL:(c + 1) * BL].reshape(TOK, H).T.copy()  # [H, TOK]
        in_maps.append(dict(shared, hT0=np.ascontiguousarray(hc)))

    t0 = time.time()
    res = run_bass_kernel_spmd(nc, in_maps, list(range(NCORES)))
    LAST_HW_NS = int((time.time() - t0) * 1e9)

    h = np.zeros((B, S, H), np.float32)
    for c in range(NCORES):
        h[c * BL:(c + 1) * BL] = res.results[c]["hTout"].T.reshape(BL, S, H)
    if DEV_LAYERS < NL:                 # debugging path: finish on host
        a2 = {k: (v[DEV_LAYERS:] if k in ("Wqkv", "bqkv", "Wo", "bo", "ln1_g",
              "ln1_b", "W1", "b1", "W2", "b2", "ln2_g", "ln2_b") else v)
              for k, v in a.items()}
        h = _bert_host(h, a2, NL - DEV_LAYERS)
    return h


def kernel(input_ids, attention_mask, labels, emb_tok, emb_pos, emb_type,
           ln_emb_g, ln_emb_b, Wqkv, bqkv, Wo, bo, ln1_g, ln1_b, W1, b1,
           W2, b2, ln2_g, ln2_b, Wih_f, Whh_f, bih_f, bhh_f, Wih_b, Whh_b,
           bih_b, bhh_b, Wc, bc, tag_weight, crf_start, crf_end, crf_trans):
    args = {k: np.asarray(v) for k, v in locals().items()}
    maskf = args['attention_mask'].astype(np.float32)

    h0 = (args['emb_tok'][args['input_ids']] + args['emb_pos'][:S][None]
          + args['emb_type'][0][None, None]).astype(np.float32)
    h0 = _ln_np(h0, args['ln_emb_g'], args['ln_emb_b'])

    h = run_device(h0, args)

    hf = _lstm_host(h, args['Wih_f'], args['Whh_f'], args['bih_f'], args['bhh_f'], False)
    hb = _lstm_host(h, args['Wih_b'], args['Whh_b'], args['bih_b'], args['bhh_b'], True)
    logits = (np.concatenate([hf, hb], -1) @ args['Wc'] + args['bc']) * args['tag_weight']
    ll = _crf_host(logits, args['labels'], maskf, args['crf_start'],
                   args['crf_end'], args['crf_trans'])
    return np.float32(ll.mean())



# revision 22
# speedup vs baseline: 14765.3510x; 14765.3510x over previous
"""BertBiLSTMCRF loss kernel for 8 Trainium2 NeuronCores.

Sharding: data-parallel over batch (B=32 -> 4 sentences/core). The BERT
encoder (>95% of FLOPs) runs on-device in raw Bass with bf16 matmuls
(fp32 PSUM accumulation, fp32 LN statistics). Activations are kept in
transposed layout hT=[H, tokens] on chip so every GEMM consumes weights
in their stored [in,out] layout as lhsT with no activation transposes;
attention computes S^T (k on partitions), uses unnormalized exp (scores
are tiny after LN + 0.02-scale weights) and gets the softmax denominator
via a ones-column matmul, so no partition-dim max/sum is ever needed.

Engine synchronization uses interval-based read/write dependency
tracking (RAW/WAR/WAW on [partition, column] boxes per buffer) instead
of conservative all-history waits, so PE/ACT/DVE/DMA overlap. Weights
stream through a 3-slot slab buffer with one DMA per output m-tile.

The BiLSTM/CRF tail (small FLOPs, serial scans) runs on host, as does
the embedding gather.
"""
import os
import sys
import types
import numpy as np
import ml_dtypes
from scipy.special import erf

V, H, NL, NH, S, B, HL, T = 30522, 768, 12, 12, 256, 32, 256, 9
DH = H // NH
FF = 4 * H
NCORES = 8
BL = B // NCORES          # sentences per core
TOK = BL * S              # tokens per core (1024)
KT = H // 128             # 6 k-tiles over hidden
USE_DEVICE = os.environ.get("KERNEL_HOST", "") == ""
DEV_LAYERS = int(os.environ.get("KERNEL_LAYERS", str(NL)))
TRACE = os.environ.get("KERNEL_TRACE", "") == "1"

LAST_HW_NS = None
_CACHE = {}
BF16 = ml_dtypes.bfloat16


def _install_ntff_shim():
    """The agent image's antenv lacks axon_hooks, which bass_utils
    imports for trace=True under axon. Shim it and install the ctypes
    NTFF hook so exec_time_ns can be measured."""
    if "antenv.axon_hooks" in sys.modules:
        return
    try:
        mod = types.ModuleType("antenv.axon_hooks")
        mod._hook = None

        def set_axon_ntff_profile_hook(h):
            mod._hook = h

        def get_axon_ntff_profile_hook():
            return mod._hook

        mod.set_axon_ntff_profile_hook = set_axon_ntff_profile_hook
        mod.get_axon_ntff_profile_hook = get_axon_ntff_profile_hook
        sys.modules["antenv.axon_hooks"] = mod
        import antenv
        antenv.axon_hooks = mod
        from trn_agent_boot.trn_boot import _ntff_profile_via_ctypes
        mod.set_axon_ntff_profile_hook(
            _ntff_profile_via_ctypes('/opt/axon/libaxon_pjrt.so'))
    except Exception:
        pass


# ---------------------------------------------------------------- host math
def _ln_np(x, g, b):
    m = x.mean(-1, keepdims=True)
    v = ((x - m) ** 2).mean(-1, keepdims=True)
    return (x - m) / np.sqrt(v + 1e-12) * g + b


SIMACT = os.environ.get("KERNEL_SIMACT", "") == "1"   # CoreSim lacks Gelu


def _gelu_np(x):
    if SIMACT:
        return np.tanh(x).astype(np.float32)
    return (0.5 * x * (1.0 + erf(x / np.float32(np.sqrt(2.0))))).astype(np.float32)


def _sigmoid_np(x):
    return 1.0 / (1.0 + np.exp(-x))


def _bert_host(h, a, n_layers=NL):
    Bc = h.shape[0]
    for l in range(n_layers):
        qkv = h @ a['Wqkv'][l] + a['bqkv'][l]
        q, k, v = [t.reshape(Bc, S, NH, DH) for t in np.split(qkv, 3, axis=-1)]
        sc = np.einsum('bqhd,bkhd->bhqk', q, k) / np.float32(np.sqrt(DH))
        sc = sc - sc.max(-1, keepdims=True)
        p = np.exp(sc)
        p = p / p.sum(-1, keepdims=True)
        ctx = np.einsum('bhqk,bkhd->bqhd', p, v).reshape(Bc, S, H)
        h = _ln_np(h + ctx @ a['Wo'][l] + a['bo'][l], a['ln1_g'][l], a['ln1_b'][l])
        ff = _gelu_np(h @ a['W1'][l] + a['b1'][l]) @ a['W2'][l] + a['b2'][l]
        h = _ln_np(h + ff, a['ln2_g'][l], a['ln2_b'][l])
    return h


def _lstm_host(x, Wih, Whh, bih, bhh, reverse):
    Bc = x.shape[0]
    pre = np.swapaxes(x, 0, 1) @ Wih.T + (bih + bhh)  # [S,B,4H]
    hs = np.zeros((S, Bc, HL), np.float32)
    h = np.zeros((Bc, HL), np.float32)
    c = np.zeros((Bc, HL), np.float32)
    order = range(S - 1, -1, -1) if reverse else range(S)
    WhhT = np.ascontiguousarray(Whh.T)
    for t in order:
        g = pre[t] + h @ WhhT
        i, f, gg, o = np.split(g, 4, axis=-1)
        c = _sigmoid_np(f) * c + _sigmoid_np(i) * np.tanh(gg)
        h = _sigmoid_np(o) * np.tanh(c)
        hs[t] = h
    return np.swapaxes(hs, 0, 1)


def _logsumexp(a, axis):
    m = a.max(axis=axis, keepdims=True)
    return (np.log(np.exp(a - m).sum(axis=axis, keepdims=True)) + m).squeeze(axis)


def _crf_host(logits, labels, maskf, crf_start, crf_end, crf_trans):
    em = np.take_along_axis(logits, labels[..., None], -1)[..., 0]
    tr = crf_trans[labels[:, :-1], labels[:, 1:]]
    last_idx = maskf.sum(1).astype(np.int32) - 1
    last_tag = np.take_along_axis(labels, last_idx[:, None], 1)[:, 0]
    num = (crf_start[labels[:, 0]] + em[:, 0]
           + ((em[:, 1:] + tr) * maskf[:, 1:]).sum(1) + crf_end[last_tag])
    alpha = crf_start + logits[:, 0]
    for t in range(1, S):
        nxt = _logsumexp(alpha[:, :, None] + crf_trans[None] + logits[:, t][:, None, :], 1)
        alpha = np.where(maskf[:, t][:, None] > 0, nxt, alpha)
    den = _logsumexp(alpha + crf_end, -1)
    return den - num


# ------------------------------------------------------------ device program
class Prog:
    """Raw-Bass multi-engine program recorder with interval-based
    dependency tracking. Each op declares the [p0,p1)x[c0,c1) boxes it
    reads and writes per buffer; waits are emitted only for overlapping
    RAW / WAR / WAW hazards, deduplicated per consumer engine. Same-
    engine ordering is implicit for compute engines (in-order queues);
    DMA completions are asynchronous so DMA-DMA hazards still wait."""

    ENGINES = ("pe", "act", "dve", "dma")

    def __init__(self):
        self.ops = {e: [] for e in self.ENGINES}
        self.counts = {}
        self.seen = {e: {} for e in self.ENGINES}
        self.sem_names = {"pe", "act", "dve"}
        self.writers = {}   # buf -> [(p0,p1,c0,c1,sem,val,engine)]
        self.readers = {}   # buf -> [(p0,p1,c0,c1,sem,val,engine)]
        self.nwaits = 0

    @staticmethod
    def _ov(b1, b2):
        return b1[0] < b2[1] and b2[0] < b1[1] and b1[2] < b2[3] and b2[2] < b1[3]

    @staticmethod
    def _covers(b1, b2):
        return (b1[0] <= b2[0] and b1[1] >= b2[1]
                and b1[2] <= b2[2] and b1[3] >= b2[3])

    def emit(self, engine, fn, reads=(), writes=(), dma_sem="dmaS0"):
        sem_self = dma_sem if engine == "dma" else engine
        inc = 16 if engine == "dma" else 1
        deps = {}

        def add_dep(sem, val, dep_eng):
            if dep_eng == engine and engine != "dma":
                return                      # in-order compute queue
            if val > deps.get(sem, 0):
                deps[sem] = val

        if engine == "dma":
            self.sem_names.add(sem_self)
            # DMA completions are unordered across in-flight transfers.
            # Keep at most ONE in flight per semaphore: the SP waits for
            # the previous transfer on this sem before posting, so a
            # consumer waiting an intermediate value is sound.
            prev = self.counts.get(sem_self, 0)
            if prev > 0:
                add_dep(sem_self, prev, "dma-prev")

        for box in reads:
            for w in self.writers.get(box[0], ()):
                if self._ov(box[1:], w[:4]):
                    add_dep(w[4], w[5], w[6])
        for box in writes:
            for w in self.writers.get(box[0], ()):
                if self._ov(box[1:], w[:4]):
                    add_dep(w[4], w[5], w[6])
            for r in self.readers.get(box[0], ()):
                if self._ov(box[1:], r[:4]):
                    add_dep(r[4], r[5], r[6])

        waits = []
        for sem, val in deps.items():
            if self.seen[engine].get(sem, 0) < val:
                waits.append((sem, val))
                self.seen[engine][sem] = val
        self.nwaits += len(waits)

        self.counts[sem_self] = self.counts.get(sem_self, 0) + inc
        val_self = self.counts[sem_self]
        for box in writes:
            lst = self.writers.setdefault(box[0], [])
            lst[:] = [w for w in lst if not self._covers(box[1:], w[:4])]
            lst.append((*box[1:], sem_self, val_self, engine))
            # a write also invalidates reader entries it covers from the
            # same engine+older ops is unsafe to drop; keep readers,
            # prune only exact-duplicate boxes from this engine
        for box in reads:
            lst = self.readers.setdefault(box[0], [])
            lst[:] = [r for r in lst
                      if not (r[6] == engine and self._covers(box[1:], r[:4]))]
            lst.append((*box[1:], sem_self, val_self, engine))
        self.ops[engine].append((waits, fn, sem_self, inc))

    def replay(self, engine, eng, sems):
        for waits, fn, sem_self, inc in self.ops[engine]:
            for name, val in waits:
                eng.wait_ge(sems[name], val)
            fn().then_inc(sems[sem_self], inc)


def _build_encoder(n_layers):
    import concourse.bass as bass
    import concourse.mybir as mybir
    from contextlib import ExitStack
    dt = mybir.dt
    f32 = dt.float32
    bf16 = dt.bfloat16
    AF = mybir.ActivationFunctionType
    ALU = mybir.AluOpType
    AF_GELU = AF.Tanh if SIMACT else AF.Gelu

    nc = bass.Bass()
    ctx = ExitStack()

    # ---- DRAM parameters (weights in bf16, biases/stats in f32)
    hT0 = nc.declare_dram_parameter("hT0", [H, TOK], bf16, isOutput=False)
    Wqkv = nc.declare_dram_parameter("Wqkv", [NL, H, 3 * H], bf16, isOutput=False)
    Wo = nc.declare_dram_parameter("Wo", [NL, H, H], bf16, isOutput=False)
    W1 = nc.declare_dram_parameter("W1", [NL, H, FF], bf16, isOutput=False)
    W2 = nc.declare_dram_parameter("W2", [NL, FF, H], bf16, isOutput=False)
    biasall = nc.declare_dram_parameter("biasall", [NL, 128, 80], f32, isOutput=False)
    consts = nc.declare_dram_parameter("consts", [128, 1024], bf16, isOutput=False)
    hTout = nc.declare_dram_parameter("hTout", [H, TOK], bf16, isOutput=True)
    zscr = nc.dram_tensor("zscr", [4, 3072], f32)

    # ---- on-chip tensors
    def sbt(nm, shape, dtype=bf16):
        return ctx.enter_context(nc.sbuf_tensor(nm, shape, dtype))

    hT = sbt("hT", [128, KT, TOK])
    h1T = sbt("h1T", [128, KT, TOK])
    ctxT = sbt("ctxT", [128, KT * TOK])   # flat; viewed [128, 6, 1024]
    big = sbt("bigb", [128, 12, TOK])     # qkT in attn; ff1 0-11; LN sq
    vbuf = sbt("vbuf", [128, KT * TOK])   # v=[128tok,768f]x8 / ff1 18-23
    wsl = sbt("wsl", [128, 3, 3072])      # weight slab, 3 slots
    vwsl = sbt("vwsl", [128, KT, H])      # Wv slab
    bias = sbt("biassb", [128, 2, 80], f32)   # double-buffered per layer
    csts = sbt("csts", [128, 1024])
    stats = sbt("stats", [1, 2048], f32)  # mean | E2/var/istd
    stats2 = sbt("stats2", [1, 1024], f32)
    statsbf = sbt("statsbf", [1, 2048])   # bf16 mean | istd for broadcast
    zbuf4 = sbt("zbuf4", [97, 3072], f32)  # Z at partition bases 0/32/64/96
    zbuf = sbt("zbuf", [12, 1024], f32)   # Z regrouped per head
    zbufb = sbt("zbufb", [12, 1024])      # bf16 1/Z
    expS = sbt("expS", [128, 2, 2, S])    # parity x k-half x q

    psA = ctx.enter_context(nc.psum_tensor("psA", [128, 1024], f32))
    psB = ctx.enter_context(nc.psum_tensor("psB", [128, 1024], f32))
    pS0 = ctx.enter_context(nc.psum_tensor("pS0", [128, 2, S], f32))
    pS1 = ctx.enter_context(nc.psum_tensor("pS1", [128, 2, S], f32))
    pC0 = ctx.enter_context(nc.psum_tensor("pC0", [128, S], f32))
    pC1 = ctx.enter_context(nc.psum_tensor("pC1", [128, S], f32))

    ctxTv = ctxT[:, :].rearrange("p (n t) -> p n t", t=TOK)

    P = Prog()

    def dma(dst, src, reads=(), writes=(), sem="dmaB"):
        P.emit("dma", lambda d=dst, s=src: nc.sync.dma_start(out=d, in_=s),
               reads=reads, writes=writes, dma_sem=sem)

    def mm(out, lhsT, rhs, start, stop, reads=(), writes=()):
        P.emit("pe", lambda o=out, l=lhsT, r=rhs, a=start, b=stop:
               nc.tensor.matmul(o, l, r, start=a, stop=b),
               reads=reads, writes=writes)

    def act(out, in_, func, b=0.0, scale=1.0, reads=(), writes=()):
        P.emit("act", lambda o=out, i=in_, f=func, bb=b, s=scale:
               nc.scalar.activation(o, i, f, bias=bb, scale=s),
               reads=reads, writes=writes)

    def dve_tt(out, in0, in1, op, reads=(), writes=()):
        P.emit("dve", lambda o=out, x=in0, y=in1, z=op:
               nc.vector.tensor_tensor(o, x, y, z), reads=reads, writes=writes)

    def dve_ts(out, in_, s1, s2, op0, op1, reads=(), writes=()):
        P.emit("dve", lambda o=out, i=in_, a=s1, b=s2, x=op0, y=op1:
               nc.vector.tensor_scalar(o, i, a, b, x, y),
               reads=reads, writes=writes)

    # ---- box helpers: (buf, p0, p1, c0, c1) in each buffer's flat cols
    def bx(name, p0, p1, c0, c1):
        return (name, p0, p1, c0, c1)

    def hT_b(kt, c0=0, c1=TOK):
        return bx("hT", 0, 128, kt * TOK + c0, kt * TOK + c1)

    def h1T_b(kt, c0=0, c1=TOK):
        return bx("h1T", 0, 128, kt * TOK + c0, kt * TOK + c1)

    def ctx_b(n, c0=0, c1=TOK, p0=0, p1=128):
        return bx("ctxT", p0, p1, n * TOK + c0, n * TOK + c1)

    def big_b(n, c0=0, c1=TOK, p0=0, p1=128):
        return bx("big", p0, p1, n * TOK + c0, n * TOK + c1)

    def v_b(c0, c1, p0=0, p1=128):
        return bx("vbuf", p0, p1, c0, c1)

    def wsl_b(slot, c0=0, c1=3072):
        return bx("wsl", 0, 128, slot * 3072 + c0, slot * 3072 + c1)

    def psA_b(c0=0, c1=1024, p0=0, p1=128):
        return bx("psA", p0, p1, c0, c1)

    def psB_b(c0=0, c1=1024, p0=0, p1=128):
        return bx("psB", p0, p1, c0, c1)

    # ---- boot: constants + initial activations
    dma(csts[:, :], consts[:, :], writes=(bx("csts", 0, 128, 0, 1024),))
    dma(hT[:, :, :], hT0.rearrange("(n p) t -> p n t", p=128),
        writes=(bx("hT", 0, 128, 0, KT * TOK),))
    ones128 = csts[:, 0:1]            # bf16 ones column
    onesrow = csts[0:1, 2:130]        # bf16 [1,128] ones on partition 0
    CSTS_R = (bx("csts", 0, 128, 0, 1024),)

    def m12(m):                       # [12, 128] head-broadcast map k-tile
        return csts[0:12, 130 + m * 128:130 + (m + 1) * 128]

    def stream_gemm(W_slab_fn, n_in, n_out, rhs_tile_fn, rhs_box_fn,
                    out_fn, out_box_fn, bias_col_fn, act_fn, lparity):
        """out[m] = act(sum_kt W[kt,m].T @ rhs[kt] + bias[m]); one slab
        DMA per m-tile through 3 wsl slots; psA/psB alternate per m."""
        kt_n = n_in // 128
        mt_n = n_out // 128
        for m in range(mt_n):
            slot = m % 3
            wdst = wsl[:, slot, 0:kt_n * 128].rearrange(
                "p (n m) -> p n m", m=128)
            dma(wdst, W_slab_fn(m), writes=(wsl_b(slot, 0, kt_n * 128),),
                sem="dmaW%d" % slot)
            ps, ps_box = (psA, psA_b) if m % 2 == 0 else (psB, psB_b)
            for half in range(2):
                c0, c1 = half * 512, (half + 1) * 512
                for kt in range(kt_n):
                    mm(ps[:, c0:c1],
                       wsl[:, slot, kt * 128:(kt + 1) * 128],
                       rhs_tile_fn(kt)[:, c0:c1],
                       start=(kt == 0), stop=(kt == kt_n - 1),
                       reads=(wsl_b(slot, kt * 128, (kt + 1) * 128),
                              rhs_box_fn(kt, c0, c1)),
                       writes=(ps_box(c0, c1),))
            act(out_fn(m), ps[:, 0:TOK], act_fn,
                b=bias[:, lparity, bias_col_fn(m):bias_col_fn(m) + 1],
                reads=(ps_box(0, TOK),
                       bx("bias", 0, 128, lparity * 80 + bias_col_fn(m),
                          lparity * 80 + bias_col_fn(m) + 1)),
                writes=(out_box_fn(m),))

    def layernorm(x, xb, gcol0, bcol0, lparity):
        """In-place LN over features of x=[128,KT,TOK] (bf16), fp32
        stats. Uses big[:,0:KT] as square scratch."""
        # x row-sums first (no dependency on squares)
        for half in range(2):
            c0, c1 = half * 512, (half + 1) * 512
            for kt in range(KT):
                mm(psA[0:1, c0:c1], ones128, x[:, kt, c0:c1],
                   start=(kt == 0), stop=(kt == KT - 1),
                   reads=CSTS_R + (xb(kt, c0, c1),),
                   writes=(psA_b(c0, c1, 0, 1),))
        for kt in range(KT):
            act(big[:, kt, :], x[:, kt, :], AF.Square,
                reads=(xb(kt),), writes=(big_b(kt),))
        for half in range(2):
            c0, c1 = half * 512, (half + 1) * 512
            for kt in range(KT):
                mm(psB[0:1, c0:c1], ones128, big[:, kt, c0:c1],
                   start=(kt == 0), stop=(kt == KT - 1),
                   reads=CSTS_R + (big_b(kt, c0, c1),),
                   writes=(psB_b(c0, c1, 0, 1),))
        mean = stats[0:1, 0:1024]
        blk = stats[0:1, 1024:2048]       # E2 -> var -> istd, in place
        tmp = stats2[0:1, :]              # meansq -> sd
        MEAN_B = (bx("stats", 0, 1, 0, 1024),)
        BLK_B = (bx("stats", 0, 1, 1024, 2048),)
        TMP_B = (bx("stats2", 0, 1, 0, 1024),)
        act(mean, psA[0:1, :], AF.Identity, scale=1.0 / H,
            reads=(psA_b(0, 1024, 0, 1),), writes=MEAN_B)
        act(blk, psB[0:1, :], AF.Identity, scale=1.0 / H,
            reads=(psB_b(0, 1024, 0, 1),), writes=BLK_B)
        dve_tt(tmp, mean, mean, ALU.mult, reads=MEAN_B, writes=TMP_B)
        dve_tt(blk, blk, tmp, ALU.subtract, reads=BLK_B + TMP_B, writes=BLK_B)
        # istd = exp(-0.5*ln(var+eps)); AF.Rsqrt is rejected by bass and
        # a [1,1024] DVE reciprocal costs ~8us (column-serial)
        P.emit("dve", lambda: nc.vector.tensor_scalar_add(blk, blk, 1e-12),
               reads=BLK_B, writes=BLK_B)
        act(tmp, blk, AF.Ln, reads=BLK_B, writes=TMP_B)
        act(tmp, tmp, AF.Exp, scale=-0.5, reads=TMP_B, writes=TMP_B)
        # bf16 copies for the broadcast matmuls
        MEANB_B = (bx("statsbf", 0, 1, 0, 1024),)
        ISTDB_B = (bx("statsbf", 0, 1, 1024, 2048),)
        act(statsbf[0:1, 0:1024], mean, AF.Identity,
            reads=MEAN_B, writes=MEANB_B)
        act(statsbf[0:1, 1024:2048], tmp, AF.Identity,
            reads=TMP_B, writes=ISTDB_B)
        for half in range(2):
            c0, c1 = half * 512, (half + 1) * 512
            mm(psA[:, c0:c1], onesrow, statsbf[0:1, c0:c1], start=True,
               stop=True, reads=CSTS_R + MEANB_B, writes=(psA_b(c0, c1),))
            mm(psB[:, c0:c1], onesrow, statsbf[0:1, 1024 + c0:1024 + c1],
               start=True, stop=True, reads=CSTS_R + ISTDB_B,
               writes=(psB_b(c0, c1),))
        for kt in range(KT):
            dve_tt(x[:, kt, :], x[:, kt, :], psA[:, 0:TOK], ALU.subtract,
                   reads=(xb(kt), psA_b(0, TOK)), writes=(xb(kt),))
            dve_tt(x[:, kt, :], x[:, kt, :], psB[:, 0:TOK], ALU.mult,
                   reads=(xb(kt), psB_b(0, TOK)), writes=(xb(kt),))
            dve_ts(x[:, kt, :], x[:, kt, :],
                   bias[:, lparity, gcol0 + kt:gcol0 + kt + 1],
                   bias[:, lparity, bcol0 + kt:bcol0 + kt + 1],
                   ALU.mult, ALU.add,
                   reads=(xb(kt),
                          bx("bias", 0, 128, lparity * 80 + gcol0 + kt,
                             lparity * 80 + gcol0 + kt + 1),
                          bx("bias", 0, 128, lparity * 80 + bcol0 + kt,
                             lparity * 80 + bcol0 + kt + 1)),
                   writes=(xb(kt),))

    def vtile(m):                     # v token-tile m: [128, 768]
        return vbuf[:, m * H:(m + 1) * H]

    def fftile(kt):                   # ff1 feature k-tile: [128, 1024]
        if kt < 12:
            return big[:, kt, :]
        if kt < 18:
            return ctxTv[:, kt - 12, :]
        return vbuf[:, (kt - 18) * TOK:(kt - 17) * TOK]

    def fftile_b(kt, c0=0, c1=TOK):
        if kt < 12:
            return big_b(kt, c0, c1)
        if kt < 18:
            return ctx_b(kt - 12, c0, c1)
        return v_b((kt - 18) * TOK + c0, (kt - 18) * TOK + c1)

    for l in range(n_layers):
        lp = l % 2
        dma(bias[:, lp, :], biasall[l],
            writes=(bx("bias", 0, 128, lp * 80, lp * 80 + 80),))

        # qkT into big[:, 0:12]: features q(0-5) k(6-11)
        Wq_r = Wqkv[l].rearrange("(n p) m -> p n m", p=128)
        stream_gemm(lambda m: Wq_r[:, :, m * 128:(m + 1) * 128], H, 1536,
                    lambda kt: hT[:, kt, :], hT_b,
                    lambda m: big[:, m, :], big_b,
                    lambda m: m, AF.Identity, lp)

        # v = hT.T @ Wv  (token-major; bias folded in after softmax)
        dma(vwsl[:, :, :], Wq_r[:, :, 1536:2304],
            writes=(bx("vwsl", 0, 128, 0, KT * H),), sem="dmaV")
        for m in range(8):
            ps, ps_box = (psA, psA_b) if m % 2 == 0 else (psB, psB_b)
            for c0, c1 in ((0, 512), (512, 768)):
                for kt in range(KT):
                    mm(ps[:, c0:c1], hT[:, kt, m * 128:(m + 1) * 128],
                       vwsl[:, kt, c0:c1],
                       start=(kt == 0), stop=(kt == KT - 1),
                       reads=(hT_b(kt, m * 128, (m + 1) * 128),
                              bx("vwsl", 0, 128, kt * H + c0, kt * H + c1)),
                       writes=(ps_box(c0, c1),))
            act(vtile(m), ps[:, 0:H], AF.Identity,
                reads=(ps_box(0, H),), writes=(v_b(m * H, (m + 1) * H),))

        # attention: software-pipelined, parity on pS/expS/pCz
        def head_sc(t):
            s, hh = divmod(t, NH)
            par = t % 2
            pS = pS0 if par == 0 else pS1
            pSn = "pS0" if par == 0 else "pS1"
            prow = 64 * (hh % 2)
            qt = big[prow:prow + 64, hh // 2, s * S:(s + 1) * S]
            for i in range(2):
                ktap = big[prow:prow + 64,
                           6 + hh // 2, s * S + i * 128:s * S + (i + 1) * 128]
                mm(pS[:, i, :], ktap, qt, start=True, stop=True,
                   reads=(big_b(6 + hh // 2, s * S + i * 128,
                                s * S + (i + 1) * 128, prow, prow + 64),
                          big_b(hh // 2, s * S, (s + 1) * S, prow, prow + 64)),
                   writes=(bx(pSn, 0, 128, i * S, (i + 1) * S),))
            act(expS[:, par, :, :], pS[:, :, :], AF.Exp, scale=1.0 / 8.0,
                reads=(bx(pSn, 0, 128, 0, 2 * S),),
                writes=(bx("expS", 0, 128, par * 2 * S, (par + 1) * 2 * S),))

        def head_pv(t):
            s, hh = divmod(t, NH)
            par = t % 2
            pS = pS0 if par == 0 else pS1
            pSn = "pS0" if par == 0 else "pS1"
            EX_R = (bx("expS", 0, 128, par * 2 * S, (par + 1) * 2 * S),)
            pC = pC0 if par == 0 else pC1
            pCn = "pC0" if par == 0 else "pC1"
            for i in range(2):
                mm(pC[0:64, :],
                   vtile(2 * s + i)[:, hh * 64:(hh + 1) * 64],
                   expS[:, par, i, :], start=(i == 0), stop=(i == 1),
                   reads=(v_b((2 * s + i) * H + hh * 64,
                              (2 * s + i) * H + (hh + 1) * 64),) + EX_R,
                   writes=(bx(pCn, 0, 64, 0, S),))
                mm(pS[0:1, 0, 0:S], ones128, expS[:, par, i, :],
                   start=(i == 0), stop=(i == 1),
                   reads=CSTS_R + EX_R,
                   writes=(bx(pSn, 0, 1, 0, S),))
            prow = 64 * (hh % 2)
            act(ctxTv[prow:prow + 64, hh // 2, s * S:(s + 1) * S],
                pC[0:64, :], AF.Identity,
                reads=(bx(pCn, 0, 64, 0, S),),
                writes=(ctx_b(hh // 2, s * S, (s + 1) * S, prow, prow + 64),))
            zr = zbuf4[32 * (hh % 4):32 * (hh % 4) + 1,
                       (hh // 4) * 1024 + s * S:(hh // 4) * 1024 + (s + 1) * S]
            act(zr, pS[0:1, 0, 0:S], AF.Identity,
                reads=(bx(pSn, 0, 1, 0, S),),
                writes=(bx("zbuf4", 32 * (hh % 4), 32 * (hh % 4) + 1,
                           (hh // 4) * 1024 + s * S,
                           (hh // 4) * 1024 + (s + 1) * S),))

        head_sc(0)
        for t in range(1, BL * NH):
            head_sc(t)
            head_pv(t - 1)
        head_pv(BL * NH - 1)

        # normalize ctx by Z (per head), add v bias. Z sums land in zbuf4
        # rows (partition bases 0/32/64/96); one SBUF->SBUF DMA regroups
        # them to [12, 1024], then a single 12-partition-parallel
        # reciprocal emits bf16 1/Z directly.
        dma(zscr[:, :], zbuf4[0:97:32, :],
            reads=(bx("zbuf4", 0, 97, 0, 3072),),
            writes=(bx("zscr", 0, 4, 0, 3072),), sem="dmaZ")
        dma(zbuf[0:12, :], zscr[:, :].rearrange("p (b t) -> (p b) t", b=3),
            reads=(bx("zscr", 0, 4, 0, 3072),),
            writes=(bx("zbuf", 0, 12, 0, 1024),), sem="dmaZ")
        def _recip_z():
            with nc.allow_low_precision(reason="1/Z feeds bf16 matmul"):
                return nc.vector.reciprocal(zbufb[0:12, :], zbuf[0:12, :])
        P.emit("dve", _recip_z,
               reads=(bx("zbuf", 0, 12, 0, 1024),),
               writes=(bx("zbufb", 0, 12, 0, 1024),))
        for m in range(KT):
            for half in range(2):
                c0, c1 = half * 512, (half + 1) * 512
                mm(psA[:, c0:c1], m12(m), zbufb[0:12, c0:c1], start=True,
                   stop=True,
                   reads=CSTS_R + (bx("zbufb", 0, 12, c0, c1),),
                   writes=(psA_b(c0, c1),))
            dve_tt(ctxTv[:, m, :], ctxTv[:, m, :], psA[:, 0:TOK], ALU.mult,
                   reads=(ctx_b(m), psA_b(0, TOK)), writes=(ctx_b(m),))
            P.emit("dve", lambda m=m, lp=lp: nc.vector.tensor_scalar_add(
                ctxTv[:, m, :], ctxTv[:, m, :], bias[:, lp, 12 + m:13 + m]),
                reads=(ctx_b(m), bx("bias", 0, 128, lp * 80 + 12 + m,
                                    lp * 80 + 13 + m)),
                writes=(ctx_b(m),))

        # attn proj + residual + LN1
        Wo_r = Wo[l].rearrange("(n p) m -> p n m", p=128)
        stream_gemm(lambda m: Wo_r[:, :, m * 128:(m + 1) * 128], H, H,
                    lambda kt: ctxTv[:, kt, :], ctx_b,
                    lambda m: h1T[:, m, :], h1T_b,
                    lambda m: 18 + m, AF.Identity, lp)
        for m in range(KT):
            dve_tt(h1T[:, m, :], h1T[:, m, :], hT[:, m, :], ALU.add,
                   reads=(h1T_b(m), hT_b(m)), writes=(h1T_b(m),))
        layernorm(h1T, h1T_b, 24, 30, lp)

        # FF1 (gelu) into big/ctxT/vbuf tiles
        W1_r = W1[l].rearrange("(n p) m -> p n m", p=128)
        stream_gemm(lambda m: W1_r[:, :, m * 128:(m + 1) * 128], H, FF,
                    lambda kt: h1T[:, kt, :], h1T_b,
                    fftile, fftile_b,
                    lambda m: 36 + m, AF_GELU, lp)

        # FF2 + residual + LN2 -> hT
        W2_r = W2[l].rearrange("(n p) m -> p n m", p=128)
        stream_gemm(lambda m: W2_r[:, :, m * 128:(m + 1) * 128], FF, H,
                    fftile, fftile_b,
                    lambda m: hT[:, m, :], hT_b,
                    lambda m: 60 + m, AF.Identity, lp)
        for m in range(KT):
            dve_tt(hT[:, m, :], hT[:, m, :], h1T[:, m, :], ALU.add,
                   reads=(hT_b(m), h1T_b(m)), writes=(hT_b(m),))
        layernorm(hT, hT_b, 66, 72, lp)

    dma(hTout.rearrange("(n p) t -> p n t", p=128), hT[:, :, :],
        reads=(bx("hT", 0, 128, 0, KT * TOK),),
        writes=(bx("hTout", 0, 128, 0, KT * TOK),))

    # ---- replay into engine blocks
    sems = {}
    for name in sorted(P.sem_names):
        sems[name] = ctx.enter_context(nc.semaphore(name))
    with nc.Block() as block:
        @block.tensor
        def _(eng):
            P.replay("pe", eng, sems)

        @block.scalar
        def _(eng):
            P.replay("act", eng, sems)

        @block.vector
        def _(eng):
            P.replay("dve", eng, sems)

        @block.sync
        def _(eng):
            P.replay("dma", eng, sems)

    return nc, ctx


def _pack_consts():
    c = np.zeros((128, 1024), np.float32)
    c[:, 0] = 1.0                       # ones128
    c[0, 2:130] = 1.0                   # onesrow
    # zbuf row r (after the strided reshape DMA) holds head (r%3)*4 + r//3
    for r in range(NH):
        hh = (r % 3) * 4 + r // 3
        for f in range(H):
            if f // DH == hh:
                c[r, 130 + f] = 1.0
    return c.astype(BF16)


def _pack_bias(a):
    out = np.zeros((NL, 128, 80), np.float32)

    def col(vec):                       # feature vec [n*128] -> [128, n]
        return vec.reshape(-1, 128).T

    for l in range(NL):
        out[l, :, 0:18] = col(a['bqkv'][l])
        out[l, :, 18:24] = col(a['bo'][l])
        out[l, :, 24:30] = col(a['ln1_g'][l])
        out[l, :, 30:36] = col(a['ln1_b'][l])
        out[l, :, 36:60] = col(a['b1'][l])
        out[l, :, 60:66] = col(a['b2'][l])
        out[l, :, 66:72] = col(a['ln2_g'][l])
        out[l, :, 72:78] = col(a['ln2_b'][l])
    return out


def run_device(h0, a):
    global LAST_HW_NS
    if not USE_DEVICE:
        return _bert_host(h0, a)
    import time
    if TRACE:
        _install_ntff_shim()
    from concourse.bass_utils import run_bass_kernel_spmd

    key = ("enc", DEV_LAYERS)
    if key not in _CACHE:
        _CACHE[key] = _build_encoder(DEV_LAYERS)
    nc, _ctx = _CACHE[key]

    biasall = _pack_bias(a)
    consts = _pack_consts()
    shared = {"Wqkv": a['Wqkv'].astype(BF16), "Wo": a['Wo'].astype(BF16),
              "W1": a['W1'].astype(BF16), "W2": a['W2'].astype(BF16),
              "biasall": biasall, "consts": consts}
    in_maps = []
    for c in range(NCORES):
        hc = h0[c * BL:(c + 1) * BL].reshape(TOK, H).T
        in_maps.append(dict(shared, hT0=np.ascontiguousarray(hc).astype(BF16)))

    t0 = time.time()
    res = run_bass_kernel_spmd(nc, in_maps, list(range(NCORES)), trace=TRACE)
    wall_ns = int((time.time() - t0) * 1e9)
    LAST_HW_NS = res.exec_time_ns if res.exec_time_ns else wall_ns

    h = np.zeros((B, S, H), np.float32)
    for c in range(NCORES):
        h[c * BL:(c + 1) * BL] = \
            res.results[c]["hTout"].astype(np.float32).T.reshape(BL, S, H)
    if DEV_LAYERS < NL:                 # debugging path: finish on host
        a2 = {k: (v[DEV_LAYERS:] if k in ("Wqkv", "bqkv", "Wo", "bo", "ln1_g",
              "ln1_b", "W1", "b1", "W2", "b2", "ln2_g", "ln2_b") else v)
              for k, v in a.items()}
        h = _bert_host(h, a2, NL - DEV_LAYERS)
    return h


def kernel(input_ids, attention_mask, labels, emb_tok, emb_pos, emb_type,
           ln_emb_g, ln_emb_b, Wqkv, bqkv, Wo, bo, ln1_g, ln1_b, W1, b1,
           W2, b2, ln2_g, ln2_b, Wih_f, Whh_f, bih_f, bhh_f, Wih_b, Whh_b,
           bih_b, bhh_b, Wc, bc, tag_weight, crf_start, crf_end, crf_trans):
    args = {k: np.asarray(v) for k, v in locals().items()}
    maskf = args['attention_mask'].astype(np.float32)

    h0 = (args['emb_tok'][args['input_ids']] + args['emb_pos'][:S][None]
          + args['emb_type'][0][None, None]).astype(np.float32)
    h0 = _ln_np(h0, args['ln_emb_g'], args['ln_emb_b'])

    h = run_device(h0, args)

    hf = _lstm_host(h, args['Wih_f'], args['Whh_f'], args['bih_f'], args['bhh_f'], False)
    hb = _lstm_host(h, args['Wih_b'], args['Whh_b'], args['bih_b'], args['bhh_b'], True)
    logits = (np.concatenate([hf, hb], -1) @ args['Wc'] + args['bc']) * args['tag_weight']
    ll = _crf_host(logits, args['labels'], maskf, args['crf_start'],
                   args['crf_end'], args['crf_trans'])
    return np.float32(ll.mean())


# revision 23
# speedup vs baseline: 15179.6447x; 1.0281x over previous
"""BertBiLSTMCRF loss kernel for 8 Trainium2 NeuronCores.

Sharding: data-parallel over batch (B=32 -> 4 sentences/core). The BERT
encoder (>95% of FLOPs) runs on-device in raw Bass with bf16 matmuls
(fp32 PSUM accumulation, fp32 LN statistics). Activations are kept in
transposed layout hT=[H, tokens] on chip so every GEMM consumes weights
in their stored [in,out] layout as lhsT with no activation transposes;
attention computes S^T (k on partitions), uses unnormalized exp (scores
are tiny after LN + 0.02-scale weights) and gets the softmax denominator
via a ones-column matmul, so no partition-dim max/sum is ever needed.

Engine synchronization uses interval-based read/write dependency
tracking (RAW/WAR/WAW on [partition, column] boxes per buffer) instead
of conservative all-history waits, so PE/ACT/DVE/DMA overlap. Weights
stream through a 3-slot slab buffer with one DMA per output m-tile.

The BiLSTM/CRF tail (small FLOPs, serial scans) runs on host, as does
the embedding gather.
"""
import os
import sys
import types
import numpy as np
import ml_dtypes
from scipy.special import erf

V, H, NL, NH, S, B, HL, T = 30522, 768, 12, 12, 256, 32, 256, 9
DH = H // NH
FF = 4 * H
NCORES = 8
BL = B // NCORES          # sentences per core
TOK = BL * S              # tokens per core (1024)
KT = H // 128             # 6 k-tiles over hidden
USE_DEVICE = os.environ.get("KERNEL_HOST", "") == ""
DEV_LAYERS = int(os.environ.get("KERNEL_LAYERS", str(NL)))
TRACE = os.environ.get("KERNEL_TRACE", "") == "1"

LAST_HW_NS = None
_CACHE = {}
BF16 = ml_dtypes.bfloat16


def _install_ntff_shim():
    """The agent image's antenv lacks axon_hooks, which bass_utils
    imports for trace=True under axon. Shim it and install the ctypes
    NTFF hook so exec_time_ns can be measured."""
    if "antenv.axon_hooks" in sys.modules:
        return
    try:
        mod = types.ModuleType("antenv.axon_hooks")
        mod._hook = None

        def set_axon_ntff_profile_hook(h):
            mod._hook = h

        def get_axon_ntff_profile_hook():
            return mod._hook

        mod.set_axon_ntff_profile_hook = set_axon_ntff_profile_hook
        mod.get_axon_ntff_profile_hook = get_axon_ntff_profile_hook
        sys.modules["antenv.axon_hooks"] = mod
        import antenv
        antenv.axon_hooks = mod
        from trn_agent_boot.trn_boot import _ntff_profile_via_ctypes
        mod.set_axon_ntff_profile_hook(
            _ntff_profile_via_ctypes('/opt/axon/libaxon_pjrt.so'))
    except Exception:
        pass


# ---------------------------------------------------------------- host math
def _ln_np(x, g, b):
    m = x.mean(-1, keepdims=True)
    v = ((x - m) ** 2).mean(-1, keepdims=True)
    return (x - m) / np.sqrt(v + 1e-12) * g + b


SIMACT = os.environ.get("KERNEL_SIMACT", "") == "1"   # CoreSim lacks Gelu


def _gelu_np(x):
    if SIMACT:
        return np.tanh(x).astype(np.float32)
    return (0.5 * x * (1.0 + erf(x / np.float32(np.sqrt(2.0))))).astype(np.float32)


def _sigmoid_np(x):
    return 1.0 / (1.0 + np.exp(-x))


def _bert_host(h, a, n_layers=NL):
    Bc = h.shape[0]
    for l in range(n_layers):
        qkv = h @ a['Wqkv'][l] + a['bqkv'][l]
        q, k, v = [t.reshape(Bc, S, NH, DH) for t in np.split(qkv, 3, axis=-1)]
        sc = np.einsum('bqhd,bkhd->bhqk', q, k) / np.float32(np.sqrt(DH))
        sc = sc - sc.max(-1, keepdims=True)
        p = np.exp(sc)
        p = p / p.sum(-1, keepdims=True)
        ctx = np.einsum('bhqk,bkhd->bqhd', p, v).reshape(Bc, S, H)
        h = _ln_np(h + ctx @ a['Wo'][l] + a['bo'][l], a['ln1_g'][l], a['ln1_b'][l])
        ff = _gelu_np(h @ a['W1'][l] + a['b1'][l]) @ a['W2'][l] + a['b2'][l]
        h = _ln_np(h + ff, a['ln2_g'][l], a['ln2_b'][l])
    return h


def _lstm_host(x, Wih, Whh, bih, bhh, reverse):
    Bc = x.shape[0]
    pre = np.swapaxes(x, 0, 1) @ Wih.T + (bih + bhh)  # [S,B,4H]
    hs = np.zeros((S, Bc, HL), np.float32)
    h = np.zeros((Bc, HL), np.float32)
    c = np.zeros((Bc, HL), np.float32)
    order = range(S - 1, -1, -1) if reverse else range(S)
    WhhT = np.ascontiguousarray(Whh.T)
    for t in order:
        g = pre[t] + h @ WhhT
        i, f, gg, o = np.split(g, 4, axis=-1)
        c = _sigmoid_np(f) * c + _sigmoid_np(i) * np.tanh(gg)
        h = _sigmoid_np(o) * np.tanh(c)
        hs[t] = h
    return np.swapaxes(hs, 0, 1)


def _logsumexp(a, axis):
    m = a.max(axis=axis, keepdims=True)
    return (np.log(np.exp(a - m).sum(axis=axis, keepdims=True)) + m).squeeze(axis)


def _crf_host(logits, labels, maskf, crf_start, crf_end, crf_trans):
    em = np.take_along_axis(logits, labels[..., None], -1)[..., 0]
    tr = crf_trans[labels[:, :-1], labels[:, 1:]]
    last_idx = maskf.sum(1).astype(np.int32) - 1
    last_tag = np.take_along_axis(labels, last_idx[:, None], 1)[:, 0]
    num = (crf_start[labels[:, 0]] + em[:, 0]
           + ((em[:, 1:] + tr) * maskf[:, 1:]).sum(1) + crf_end[last_tag])
    alpha = crf_start + logits[:, 0]
    for t in range(1, S):
        nxt = _logsumexp(alpha[:, :, None] + crf_trans[None] + logits[:, t][:, None, :], 1)
        alpha = np.where(maskf[:, t][:, None] > 0, nxt, alpha)
    den = _logsumexp(alpha + crf_end, -1)
    return den - num


# ------------------------------------------------------------ device program
class Prog:
    """Raw-Bass multi-engine program recorder with interval-based
    dependency tracking. Each op declares the [p0,p1)x[c0,c1) boxes it
    reads and writes per buffer; waits are emitted only for overlapping
    RAW / WAR / WAW hazards, deduplicated per consumer engine. Same-
    engine ordering is implicit for compute engines (in-order queues);
    DMA completions are asynchronous so DMA-DMA hazards still wait."""

    ENGINES = ("pe", "act", "dve", "dma")

    def __init__(self):
        self.ops = {e: [] for e in self.ENGINES}
        self.counts = {}
        self.seen = {e: {} for e in self.ENGINES}
        self.sem_names = {"pe", "act", "dve"}
        self.writers = {}   # buf -> [(p0,p1,c0,c1,sem,val,engine)]
        self.readers = {}   # buf -> [(p0,p1,c0,c1,sem,val,engine)]
        self.nwaits = 0

    @staticmethod
    def _ov(b1, b2):
        return b1[0] < b2[1] and b2[0] < b1[1] and b1[2] < b2[3] and b2[2] < b1[3]

    @staticmethod
    def _covers(b1, b2):
        return (b1[0] <= b2[0] and b1[1] >= b2[1]
                and b1[2] <= b2[2] and b1[3] >= b2[3])

    def emit(self, engine, fn, reads=(), writes=(), dma_sem="dmaS0"):
        sem_self = dma_sem if engine == "dma" else engine
        inc = 16 if engine == "dma" else 1
        deps = {}

        def add_dep(sem, val, dep_eng):
            if dep_eng == engine and engine != "dma":
                return                      # in-order compute queue
            if val > deps.get(sem, 0):
                deps[sem] = val

        if engine == "dma":
            self.sem_names.add(sem_self)
            # DMA completions are unordered across in-flight transfers.
            # Keep at most ONE in flight per semaphore: the SP waits for
            # the previous transfer on this sem before posting, so a
            # consumer waiting an intermediate value is sound.
            prev = self.counts.get(sem_self, 0)
            if prev > 0:
                add_dep(sem_self, prev, "dma-prev")

        for box in reads:
            for w in self.writers.get(box[0], ()):
                if self._ov(box[1:], w[:4]):
                    add_dep(w[4], w[5], w[6])
        for box in writes:
            for w in self.writers.get(box[0], ()):
                if self._ov(box[1:], w[:4]):
                    add_dep(w[4], w[5], w[6])
            for r in self.readers.get(box[0], ()):
                if self._ov(box[1:], r[:4]):
                    add_dep(r[4], r[5], r[6])

        waits = []
        for sem, val in deps.items():
            if self.seen[engine].get(sem, 0) < val:
                waits.append((sem, val))
                self.seen[engine][sem] = val
        self.nwaits += len(waits)

        self.counts[sem_self] = self.counts.get(sem_self, 0) + inc
        val_self = self.counts[sem_self]
        for box in writes:
            lst = self.writers.setdefault(box[0], [])
            lst[:] = [w for w in lst if not self._covers(box[1:], w[:4])]
            lst.append((*box[1:], sem_self, val_self, engine))
            # a write also invalidates reader entries it covers from the
            # same engine+older ops is unsafe to drop; keep readers,
            # prune only exact-duplicate boxes from this engine
        for box in reads:
            lst = self.readers.setdefault(box[0], [])
            lst[:] = [r for r in lst
                      if not (r[6] == engine and self._covers(box[1:], r[:4]))]
            lst.append((*box[1:], sem_self, val_self, engine))
        self.ops[engine].append((waits, fn, sem_self, inc))

    def replay(self, engine, eng, sems):
        for waits, fn, sem_self, inc in self.ops[engine]:
            for name, val in waits:
                eng.wait_ge(sems[name], val)
            fn().then_inc(sems[sem_self], inc)


def _build_encoder(n_layers):
    import concourse.bass as bass
    import concourse.mybir as mybir
    from contextlib import ExitStack
    dt = mybir.dt
    f32 = dt.float32
    bf16 = dt.bfloat16
    AF = mybir.ActivationFunctionType
    ALU = mybir.AluOpType
    AF_GELU = AF.Tanh if SIMACT else AF.Gelu

    nc = bass.Bass()
    ctx = ExitStack()

    # ---- DRAM parameters (weights in bf16, biases/stats in f32)
    hT0 = nc.declare_dram_parameter("hT0", [H, TOK], bf16, isOutput=False)
    Wqkv = nc.declare_dram_parameter("Wqkv", [NL, H, 3 * H], bf16, isOutput=False)
    Wo = nc.declare_dram_parameter("Wo", [NL, H, H], bf16, isOutput=False)
    W1 = nc.declare_dram_parameter("W1", [NL, H, FF], bf16, isOutput=False)
    W2 = nc.declare_dram_parameter("W2", [NL, FF, H], bf16, isOutput=False)
    biasall = nc.declare_dram_parameter("biasall", [NL, 128, 80], f32, isOutput=False)
    consts = nc.declare_dram_parameter("consts", [128, 1024], bf16, isOutput=False)
    hTout = nc.declare_dram_parameter("hTout", [H, TOK], bf16, isOutput=True)
    zscr = nc.dram_tensor("zscr", [4, 3072], f32)

    # ---- on-chip tensors
    def sbt(nm, shape, dtype=bf16):
        return ctx.enter_context(nc.sbuf_tensor(nm, shape, dtype))

    hT = sbt("hT", [128, KT, TOK])
    h1T = sbt("h1T", [128, KT, TOK])
    ctxT = sbt("ctxT", [128, KT * TOK])   # flat; viewed [128, 6, 1024]
    big = sbt("bigb", [128, 12, TOK])     # qkT in attn; ff1 0-11; LN sq
    vbuf = sbt("vbuf", [128, KT * TOK])   # v=[128tok,768f]x8 / ff1 18-23
    wsl = sbt("wsl", [128, 3, 3072])      # weight slab, 3 slots
    vwsl = sbt("vwsl", [128, KT, H])      # Wv slab
    bias = sbt("biassb", [128, 2, 80], f32)   # double-buffered per layer
    csts = sbt("csts", [128, 1024])
    stats = sbt("stats", [1, 2048], f32)  # mean | E2/var/istd
    stats2 = sbt("stats2", [1, 1024], f32)
    statsbf = sbt("statsbf", [1, 2048])   # bf16 mean | istd for broadcast
    zbuf4 = sbt("zbuf4", [97, 3072], f32)  # Z at partition bases 0/32/64/96
    zbuf = sbt("zbuf", [12, 1024], f32)   # Z regrouped per head
    zbufb = sbt("zbufb", [12, 1024])      # bf16 1/Z
    lnb = sbt("lnb", [128, 2, 1024])      # bf16 bcast mean | istd
    zsb = sbt("zsb", [128, KT, 1024])     # bf16 bcast 1/Z per m-tile
    expS = sbt("expS", [128, 2, 2, S])    # parity x k-half x q

    psA = ctx.enter_context(nc.psum_tensor("psA", [128, 1024], f32))
    psB = ctx.enter_context(nc.psum_tensor("psB", [128, 1024], f32))
    pS0 = ctx.enter_context(nc.psum_tensor("pS0", [128, 2, S], f32))
    pS1 = ctx.enter_context(nc.psum_tensor("pS1", [128, 2, S], f32))
    pC0 = ctx.enter_context(nc.psum_tensor("pC0", [128, S], f32))
    pC1 = ctx.enter_context(nc.psum_tensor("pC1", [128, S], f32))

    ctxTv = ctxT[:, :].rearrange("p (n t) -> p n t", t=TOK)

    P = Prog()

    def dma(dst, src, reads=(), writes=(), sem="dmaB"):
        P.emit("dma", lambda d=dst, s=src: nc.sync.dma_start(out=d, in_=s),
               reads=reads, writes=writes, dma_sem=sem)

    def mm(out, lhsT, rhs, start, stop, reads=(), writes=()):
        P.emit("pe", lambda o=out, l=lhsT, r=rhs, a=start, b=stop:
               nc.tensor.matmul(o, l, r, start=a, stop=b),
               reads=reads, writes=writes)

    def act(out, in_, func, b=0.0, scale=1.0, reads=(), writes=()):
        P.emit("act", lambda o=out, i=in_, f=func, bb=b, s=scale:
               nc.scalar.activation(o, i, f, bias=bb, scale=s),
               reads=reads, writes=writes)

    def dve_tt(out, in0, in1, op, reads=(), writes=()):
        P.emit("dve", lambda o=out, x=in0, y=in1, z=op:
               nc.vector.tensor_tensor(o, x, y, z), reads=reads, writes=writes)

    def dve_ts(out, in_, s1, s2, op0, op1, reads=(), writes=()):
        P.emit("dve", lambda o=out, i=in_, a=s1, b=s2, x=op0, y=op1:
               nc.vector.tensor_scalar(o, i, a, b, x, y),
               reads=reads, writes=writes)

    # ---- box helpers: (buf, p0, p1, c0, c1) in each buffer's flat cols
    def bx(name, p0, p1, c0, c1):
        return (name, p0, p1, c0, c1)

    def hT_b(kt, c0=0, c1=TOK):
        return bx("hT", 0, 128, kt * TOK + c0, kt * TOK + c1)

    def h1T_b(kt, c0=0, c1=TOK):
        return bx("h1T", 0, 128, kt * TOK + c0, kt * TOK + c1)

    def ctx_b(n, c0=0, c1=TOK, p0=0, p1=128):
        return bx("ctxT", p0, p1, n * TOK + c0, n * TOK + c1)

    def big_b(n, c0=0, c1=TOK, p0=0, p1=128):
        return bx("big", p0, p1, n * TOK + c0, n * TOK + c1)

    def v_b(c0, c1, p0=0, p1=128):
        return bx("vbuf", p0, p1, c0, c1)

    def wsl_b(slot, c0=0, c1=3072):
        return bx("wsl", 0, 128, slot * 3072 + c0, slot * 3072 + c1)

    def psA_b(c0=0, c1=1024, p0=0, p1=128):
        return bx("psA", p0, p1, c0, c1)

    def psB_b(c0=0, c1=1024, p0=0, p1=128):
        return bx("psB", p0, p1, c0, c1)

    # ---- boot: constants + initial activations
    dma(csts[:, :], consts[:, :], writes=(bx("csts", 0, 128, 0, 1024),))
    dma(hT[:, :, :], hT0.rearrange("(n p) t -> p n t", p=128),
        writes=(bx("hT", 0, 128, 0, KT * TOK),))
    ones128 = csts[:, 0:1]            # bf16 ones column
    onesrow = csts[0:1, 2:130]        # bf16 [1,128] ones on partition 0
    CSTS_R = (bx("csts", 0, 128, 0, 1024),)

    def m12(m):                       # [12, 128] head-broadcast map k-tile
        return csts[0:12, 130 + m * 128:130 + (m + 1) * 128]

    def stream_gemm(W_slab_fn, n_in, n_out, rhs_tile_fn, rhs_box_fn,
                    out_fn, out_box_fn, bias_col_fn, act_fn, lparity):
        """out[m] = act(sum_kt W[kt,m].T @ rhs[kt] + bias[m]); one slab
        DMA per m-tile through 3 wsl slots; psA/psB alternate per m."""
        kt_n = n_in // 128
        mt_n = n_out // 128
        for m in range(mt_n):
            slot = m % 3
            wdst = wsl[:, slot, 0:kt_n * 128].rearrange(
                "p (n m) -> p n m", m=128)
            dma(wdst, W_slab_fn(m), writes=(wsl_b(slot, 0, kt_n * 128),),
                sem="dmaW%d" % slot)
            ps, ps_box = (psA, psA_b) if m % 2 == 0 else (psB, psB_b)
            for half in range(2):
                c0, c1 = half * 512, (half + 1) * 512
                for kt in range(kt_n):
                    mm(ps[:, c0:c1],
                       wsl[:, slot, kt * 128:(kt + 1) * 128],
                       rhs_tile_fn(kt)[:, c0:c1],
                       start=(kt == 0), stop=(kt == kt_n - 1),
                       reads=(wsl_b(slot, kt * 128, (kt + 1) * 128),
                              rhs_box_fn(kt, c0, c1)),
                       writes=(ps_box(c0, c1),))
            act(out_fn(m), ps[:, 0:TOK], act_fn,
                b=bias[:, lparity, bias_col_fn(m):bias_col_fn(m) + 1],
                reads=(ps_box(0, TOK),
                       bx("bias", 0, 128, lparity * 80 + bias_col_fn(m),
                          lparity * 80 + bias_col_fn(m) + 1)),
                writes=(out_box_fn(m),))

    def layernorm(x, xb, gcol0, bcol0, lparity):
        """In-place LN over features of x=[128,KT,TOK] (bf16), fp32
        stats. Uses big[:,0:KT] as square scratch."""
        # x row-sums first (no dependency on squares)
        for half in range(2):
            c0, c1 = half * 512, (half + 1) * 512
            for kt in range(KT):
                mm(psA[0:1, c0:c1], ones128, x[:, kt, c0:c1],
                   start=(kt == 0), stop=(kt == KT - 1),
                   reads=CSTS_R + (xb(kt, c0, c1),),
                   writes=(psA_b(c0, c1, 0, 1),))
        for kt in range(KT):
            act(big[:, kt, :], x[:, kt, :], AF.Square,
                reads=(xb(kt),), writes=(big_b(kt),))
        for half in range(2):
            c0, c1 = half * 512, (half + 1) * 512
            for kt in range(KT):
                mm(psB[0:1, c0:c1], ones128, big[:, kt, c0:c1],
                   start=(kt == 0), stop=(kt == KT - 1),
                   reads=CSTS_R + (big_b(kt, c0, c1),),
                   writes=(psB_b(c0, c1, 0, 1),))
        mean = stats[0:1, 0:1024]
        blk = stats[0:1, 1024:2048]       # E2 -> var -> istd, in place
        tmp = stats2[0:1, :]              # meansq -> sd
        MEAN_B = (bx("stats", 0, 1, 0, 1024),)
        BLK_B = (bx("stats", 0, 1, 1024, 2048),)
        TMP_B = (bx("stats2", 0, 1, 0, 1024),)
        act(mean, psA[0:1, :], AF.Identity, scale=1.0 / H,
            reads=(psA_b(0, 1024, 0, 1),), writes=MEAN_B)
        act(blk, psB[0:1, :], AF.Identity, scale=1.0 / H,
            reads=(psB_b(0, 1024, 0, 1),), writes=BLK_B)
        dve_tt(tmp, mean, mean, ALU.mult, reads=MEAN_B, writes=TMP_B)
        dve_tt(blk, blk, tmp, ALU.subtract, reads=BLK_B + TMP_B, writes=BLK_B)
        # istd = exp(-0.5*ln(var+eps)); AF.Rsqrt is rejected by bass and
        # a [1,1024] DVE reciprocal costs ~8us (column-serial)
        P.emit("dve", lambda: nc.vector.tensor_scalar_add(blk, blk, 1e-12),
               reads=BLK_B, writes=BLK_B)
        act(tmp, blk, AF.Ln, reads=BLK_B, writes=TMP_B)
        act(tmp, tmp, AF.Exp, scale=-0.5, reads=TMP_B, writes=TMP_B)
        # bf16 copies for the broadcast matmuls
        MEANB_B = (bx("statsbf", 0, 1, 0, 1024),)
        ISTDB_B = (bx("statsbf", 0, 1, 1024, 2048),)
        act(statsbf[0:1, 0:1024], mean, AF.Identity,
            reads=MEAN_B, writes=MEANB_B)
        act(statsbf[0:1, 1024:2048], tmp, AF.Identity,
            reads=TMP_B, writes=ISTDB_B)
        for half in range(2):
            c0, c1 = half * 512, (half + 1) * 512
            mm(psA[:, c0:c1], onesrow, statsbf[0:1, c0:c1], start=True,
               stop=True, reads=CSTS_R + MEANB_B, writes=(psA_b(c0, c1),))
            mm(psB[:, c0:c1], onesrow, statsbf[0:1, 1024 + c0:1024 + c1],
               start=True, stop=True, reads=CSTS_R + ISTDB_B,
               writes=(psB_b(c0, c1),))
        # copy broadcasts to SBUF bf16: frees PSUM for the next GEMM
        # stream immediately and doubles DVE rate in the apply passes
        act(lnb[:, 0, :], psA[:, 0:TOK], AF.Identity,
            reads=(psA_b(0, TOK),), writes=(bx("lnb", 0, 128, 0, 1024),))
        act(lnb[:, 1, :], psB[:, 0:TOK], AF.Identity,
            reads=(psB_b(0, TOK),), writes=(bx("lnb", 0, 128, 1024, 2048),))
        for kt in range(KT):
            dve_tt(x[:, kt, :], x[:, kt, :], lnb[:, 0, :], ALU.subtract,
                   reads=(xb(kt), bx("lnb", 0, 128, 0, 1024)),
                   writes=(xb(kt),))
            dve_tt(x[:, kt, :], x[:, kt, :], lnb[:, 1, :], ALU.mult,
                   reads=(xb(kt), bx("lnb", 0, 128, 1024, 2048)),
                   writes=(xb(kt),))
            dve_ts(x[:, kt, :], x[:, kt, :],
                   bias[:, lparity, gcol0 + kt:gcol0 + kt + 1],
                   bias[:, lparity, bcol0 + kt:bcol0 + kt + 1],
                   ALU.mult, ALU.add,
                   reads=(xb(kt),
                          bx("bias", 0, 128, lparity * 80 + gcol0 + kt,
                             lparity * 80 + gcol0 + kt + 1),
                          bx("bias", 0, 128, lparity * 80 + bcol0 + kt,
                             lparity * 80 + bcol0 + kt + 1)),
                   writes=(xb(kt),))

    def vtile(m):                     # v token-tile m: [128, 768]
        return vbuf[:, m * H:(m + 1) * H]

    def fftile(kt):                   # ff1 feature k-tile: [128, 1024]
        if kt < 12:
            return big[:, kt, :]
        if kt < 18:
            return ctxTv[:, kt - 12, :]
        return vbuf[:, (kt - 18) * TOK:(kt - 17) * TOK]

    def fftile_b(kt, c0=0, c1=TOK):
        if kt < 12:
            return big_b(kt, c0, c1)
        if kt < 18:
            return ctx_b(kt - 12, c0, c1)
        return v_b((kt - 18) * TOK + c0, (kt - 18) * TOK + c1)

    for l in range(n_layers):
        lp = l % 2
        dma(bias[:, lp, :], biasall[l],
            writes=(bx("bias", 0, 128, lp * 80, lp * 80 + 80),))

        # qkT into big[:, 0:12]: features q(0-5) k(6-11)
        Wq_r = Wqkv[l].rearrange("(n p) m -> p n m", p=128)
        stream_gemm(lambda m: Wq_r[:, :, m * 128:(m + 1) * 128], H, 1536,
                    lambda kt: hT[:, kt, :], hT_b,
                    lambda m: big[:, m, :], big_b,
                    lambda m: m, AF.Identity, lp)

        # v = hT.T @ Wv  (token-major; bias folded in after softmax)
        dma(vwsl[:, :, :], Wq_r[:, :, 1536:2304],
            writes=(bx("vwsl", 0, 128, 0, KT * H),), sem="dmaV")
        for m in range(8):
            ps, ps_box = (psA, psA_b) if m % 2 == 0 else (psB, psB_b)
            for c0, c1 in ((0, 512), (512, 768)):
                for kt in range(KT):
                    mm(ps[:, c0:c1], hT[:, kt, m * 128:(m + 1) * 128],
                       vwsl[:, kt, c0:c1],
                       start=(kt == 0), stop=(kt == KT - 1),
                       reads=(hT_b(kt, m * 128, (m + 1) * 128),
                              bx("vwsl", 0, 128, kt * H + c0, kt * H + c1)),
                       writes=(ps_box(c0, c1),))
            act(vtile(m), ps[:, 0:H], AF.Identity,
                reads=(ps_box(0, H),), writes=(v_b(m * H, (m + 1) * H),))

        # attention: software-pipelined, parity on pS/expS/pCz
        def head_sc(t):
            s, hh = divmod(t, NH)
            par = t % 2
            pS = pS0 if par == 0 else pS1
            pSn = "pS0" if par == 0 else "pS1"
            prow = 64 * (hh % 2)
            qt = big[prow:prow + 64, hh // 2, s * S:(s + 1) * S]
            for i in range(2):
                ktap = big[prow:prow + 64,
                           6 + hh // 2, s * S + i * 128:s * S + (i + 1) * 128]
                mm(pS[:, i, :], ktap, qt, start=True, stop=True,
                   reads=(big_b(6 + hh // 2, s * S + i * 128,
                                s * S + (i + 1) * 128, prow, prow + 64),
                          big_b(hh // 2, s * S, (s + 1) * S, prow, prow + 64)),
                   writes=(bx(pSn, 0, 128, i * S, (i + 1) * S),))
            act(expS[:, par, :, :], pS[:, :, :], AF.Exp, scale=1.0 / 8.0,
                reads=(bx(pSn, 0, 128, 0, 2 * S),),
                writes=(bx("expS", 0, 128, par * 2 * S, (par + 1) * 2 * S),))

        def head_pv(t):
            s, hh = divmod(t, NH)
            par = t % 2
            pS = pS0 if par == 0 else pS1
            pSn = "pS0" if par == 0 else "pS1"
            EX_R = (bx("expS", 0, 128, par * 2 * S, (par + 1) * 2 * S),)
            pC = pC0 if par == 0 else pC1
            pCn = "pC0" if par == 0 else "pC1"
            for i in range(2):
                mm(pC[0:64, :],
                   vtile(2 * s + i)[:, hh * 64:(hh + 1) * 64],
                   expS[:, par, i, :], start=(i == 0), stop=(i == 1),
                   reads=(v_b((2 * s + i) * H + hh * 64,
                              (2 * s + i) * H + (hh + 1) * 64),) + EX_R,
                   writes=(bx(pCn, 0, 64, 0, S),))
                mm(pS[0:1, 0, 0:S], ones128, expS[:, par, i, :],
                   start=(i == 0), stop=(i == 1),
                   reads=CSTS_R + EX_R,
                   writes=(bx(pSn, 0, 1, 0, S),))
            prow = 64 * (hh % 2)
            act(ctxTv[prow:prow + 64, hh // 2, s * S:(s + 1) * S],
                pC[0:64, :], AF.Identity,
                reads=(bx(pCn, 0, 64, 0, S),),
                writes=(ctx_b(hh // 2, s * S, (s + 1) * S, prow, prow + 64),))
            zr = zbuf4[32 * (hh % 4):32 * (hh % 4) + 1,
                       (hh // 4) * 1024 + s * S:(hh // 4) * 1024 + (s + 1) * S]
            act(zr, pS[0:1, 0, 0:S], AF.Identity,
                reads=(bx(pSn, 0, 1, 0, S),),
                writes=(bx("zbuf4", 32 * (hh % 4), 32 * (hh % 4) + 1,
                           (hh // 4) * 1024 + s * S,
                           (hh // 4) * 1024 + (s + 1) * S),))

        head_sc(0)
        for t in range(1, BL * NH):
            head_sc(t)
            head_pv(t - 1)
        head_pv(BL * NH - 1)

        # normalize ctx by Z (per head), add v bias. Z sums land in zbuf4
        # rows (partition bases 0/32/64/96); one SBUF->SBUF DMA regroups
        # them to [12, 1024], then a single 12-partition-parallel
        # reciprocal emits bf16 1/Z directly.
        dma(zscr[:, :], zbuf4[0:97:32, :],
            reads=(bx("zbuf4", 0, 97, 0, 3072),),
            writes=(bx("zscr", 0, 4, 0, 3072),), sem="dmaZ")
        dma(zbuf[0:12, :], zscr[:, :].rearrange("p (b t) -> (p b) t", b=3),
            reads=(bx("zscr", 0, 4, 0, 3072),),
            writes=(bx("zbuf", 0, 12, 0, 1024),), sem="dmaZ")
        def _recip_z():
            with nc.allow_low_precision(reason="1/Z feeds bf16 matmul"):
                return nc.vector.reciprocal(zbufb[0:12, :], zbuf[0:12, :])
        P.emit("dve", _recip_z,
               reads=(bx("zbuf", 0, 12, 0, 1024),),
               writes=(bx("zbufb", 0, 12, 0, 1024),))
        for m in range(KT):
            ps, ps_box = (psA, psA_b) if m % 2 == 0 else (psB, psB_b)
            for half in range(2):
                c0, c1 = half * 512, (half + 1) * 512
                mm(ps[:, c0:c1], m12(m), zbufb[0:12, c0:c1], start=True,
                   stop=True,
                   reads=CSTS_R + (bx("zbufb", 0, 12, c0, c1),),
                   writes=(ps_box(c0, c1),))
            act(zsb[:, m, :], ps[:, 0:TOK], AF.Identity,
                reads=(ps_box(0, TOK),),
                writes=(bx("zsb", 0, 128, m * 1024, (m + 1) * 1024),))
            dve_tt(ctxTv[:, m, :], ctxTv[:, m, :], zsb[:, m, :], ALU.mult,
                   reads=(ctx_b(m), bx("zsb", 0, 128, m * 1024, (m + 1) * 1024)),
                   writes=(ctx_b(m),))
            P.emit("dve", lambda m=m, lp=lp: nc.vector.tensor_scalar_add(
                ctxTv[:, m, :], ctxTv[:, m, :], bias[:, lp, 12 + m:13 + m]),
                reads=(ctx_b(m), bx("bias", 0, 128, lp * 80 + 12 + m,
                                    lp * 80 + 13 + m)),
                writes=(ctx_b(m),))

        # attn proj + residual + LN1
        Wo_r = Wo[l].rearrange("(n p) m -> p n m", p=128)
        stream_gemm(lambda m: Wo_r[:, :, m * 128:(m + 1) * 128], H, H,
                    lambda kt: ctxTv[:, kt, :], ctx_b,
                    lambda m: h1T[:, m, :], h1T_b,
                    lambda m: 18 + m, AF.Identity, lp)
        for m in range(KT):
            dve_tt(h1T[:, m, :], h1T[:, m, :], hT[:, m, :], ALU.add,
                   reads=(h1T_b(m), hT_b(m)), writes=(h1T_b(m),))
        layernorm(h1T, h1T_b, 24, 30, lp)

        # FF1 (gelu) into big/ctxT/vbuf tiles
        W1_r = W1[l].rearrange("(n p) m -> p n m", p=128)
        stream_gemm(lambda m: W1_r[:, :, m * 128:(m + 1) * 128], H, FF,
                    lambda kt: h1T[:, kt, :], h1T_b,
                    fftile, fftile_b,
                    lambda m: 36 + m, AF_GELU, lp)

        # FF2 + residual + LN2 -> hT
        W2_r = W2[l].rearrange("(n p) m -> p n m", p=128)
        stream_gemm(lambda m: W2_r[:, :, m * 128:(m + 1) * 128], FF, H,
                    fftile, fftile_b,
                    lambda m: hT[:, m, :], hT_b,
                    lambda m: 60 + m, AF.Identity, lp)
        for m in range(KT):
            dve_tt(hT[:, m, :], hT[:, m, :], h1T[:, m, :], ALU.add,
                   reads=(hT_b(m), h1T_b(m)), writes=(hT_b(m),))
        layernorm(hT, hT_b, 66, 72, lp)

    dma(hTout.rearrange("(n p) t -> p n t", p=128), hT[:, :, :],
        reads=(bx("hT", 0, 128, 0, KT * TOK),),
        writes=(bx("hTout", 0, 128, 0, KT * TOK),))

    # ---- replay into engine blocks
    sems = {}
    for name in sorted(P.sem_names):
        sems[name] = ctx.enter_context(nc.semaphore(name))
    with nc.Block() as block:
        @block.tensor
        def _(eng):
            P.replay("pe", eng, sems)

        @block.scalar
        def _(eng):
            P.replay("act", eng, sems)

        @block.vector
        def _(eng):
            P.replay("dve", eng, sems)

        @block.sync
        def _(eng):
            P.replay("dma", eng, sems)

    return nc, ctx


def _pack_consts():
    c = np.zeros((128, 1024), np.float32)
    c[:, 0] = 1.0                       # ones128
    c[0, 2:130] = 1.0                   # onesrow
    # zbuf row r (after the strided reshape DMA) holds head (r%3)*4 + r//3
    for r in range(NH):
        hh = (r % 3) * 4 + r // 3
        for f in range(H):
            if f // DH == hh:
                c[r, 130 + f] = 1.0
    return c.astype(BF16)


def _pack_bias(a):
    out = np.zeros((NL, 128, 80), np.float32)

    def col(vec):                       # feature vec [n*128] -> [128, n]
        return vec.reshape(-1, 128).T

    for l in range(NL):
        out[l, :, 0:18] = col(a['bqkv'][l])
        out[l, :, 18:24] = col(a['bo'][l])
        out[l, :, 24:30] = col(a['ln1_g'][l])
        out[l, :, 30:36] = col(a['ln1_b'][l])
        out[l, :, 36:60] = col(a['b1'][l])
        out[l, :, 60:66] = col(a['b2'][l])
        out[l, :, 66:72] = col(a['ln2_g'][l])
        out[l, :, 72:78] = col(a['ln2_b'][l])
    return out


def run_device(h0, a):
    global LAST_HW_NS
    if not USE_DEVICE:
        return _bert_host(h0, a)
    import time
    if TRACE:
        _install_ntff_shim()
    from concourse.bass_utils import run_bass_kernel_spmd

    key = ("enc", DEV_LAYERS)
    if key not in _CACHE:
        _CACHE[key] = _build_encoder(DEV_LAYERS)
    nc, _ctx = _CACHE[key]

    biasall = _pack_bias(a)
    consts = _pack_consts()
    shared = {"Wqkv": a['Wqkv'].astype(BF16), "Wo": a['Wo'].astype(BF16),
              "W1": a['W1'].astype(BF16), "W2": a['W2'].astype(BF16),
              "biasall": biasall, "consts": consts}
    in_maps = []
    for c in range(NCORES):
        hc = h0[c * BL:(c + 1) * BL].reshape(TOK, H).T
        in_maps.append(dict(shared, hT0=np.ascontiguousarray(hc).astype(BF16)))

    t0 = time.time()
    res = run_bass_kernel_spmd(nc, in_maps, list(range(NCORES)), trace=TRACE)
    wall_ns = int((time.time() - t0) * 1e9)
    LAST_HW_NS = res.exec_time_ns if res.exec_time_ns else wall_ns

    h = np.zeros((B, S, H), np.float32)
    for c in range(NCORES):
        h[c * BL:(c + 1) * BL] = \
            res.results[c]["hTout"].astype(np.float32).T.reshape(BL, S, H)
    if DEV_LAYERS < NL:                 # debugging path: finish on host
        a2 = {k: (v[DEV_LAYERS:] if k in ("Wqkv", "bqkv", "Wo", "bo", "ln1_g",
              "ln1_b", "W1", "b1", "W2", "b2", "ln2_g", "ln2_b") else v)
              for k, v in a.items()}
        h = _bert_host(h, a2, NL - DEV_LAYERS)
    return h


def kernel(input_ids, attention_mask, labels, emb_tok, emb_pos, emb_type,
           ln_emb_g, ln_emb_b, Wqkv, bqkv, Wo, bo, ln1_g, ln1_b, W1, b1,
           W2, b2, ln2_g, ln2_b, Wih_f, Whh_f, bih_f, bhh_f, Wih_b, Whh_b,
           bih_b, bhh_b, Wc, bc, tag_weight, crf_start, crf_end, crf_trans):
    args = {k: np.asarray(v) for k, v in locals().items()}
    maskf = args['attention_mask'].astype(np.float32)

    h0 = (args['emb_tok'][args['input_ids']] + args['emb_pos'][:S][None]
          + args['emb_type'][0][None, None]).astype(np.float32)
    h0 = _ln_np(h0, args['ln_emb_g'], args['ln_emb_b'])

    h = run_device(h0, args)

    hf = _lstm_host(h, args['Wih_f'], args['Whh_f'], args['bih_f'], args['bhh_f'], False)
    hb = _lstm_host(h, args['Wih_b'], args['Whh_b'], args['bih_b'], args['bhh_b'], True)
    logits = (np.concatenate([hf, hb], -1) @ args['Wc'] + args['bc']) * args['tag_weight']
    ll = _crf_host(logits, args['labels'], maskf, args['crf_start'],
                   args['crf_end'], args['crf_trans'])
    return np.float32(ll.mean())


# revision 24
# speedup vs baseline: 15454.7228x; 1.0181x over previous
"""BertBiLSTMCRF loss kernel for 8 Trainium2 NeuronCores.

Sharding: data-parallel over batch (B=32 -> 4 sentences/core). The BERT
encoder (>95% of FLOPs) runs on-device in raw Bass with bf16 matmuls
(fp32 PSUM accumulation, fp32 LN statistics). Activations are kept in
transposed layout hT=[H, tokens] on chip so every GEMM consumes weights
in their stored [in,out] layout as lhsT with no activation transposes;
attention computes S^T (k on partitions), uses unnormalized exp (scores
are tiny after LN + 0.02-scale weights) and gets the softmax denominator
via a ones-column matmul, so no partition-dim max/sum is ever needed.

Engine synchronization uses interval-based read/write dependency
tracking (RAW/WAR/WAW on [partition, column] boxes per buffer) instead
of conservative all-history waits, so PE/ACT/DVE/DMA overlap. Weights
stream through a 3-slot slab buffer with one DMA per output m-tile.

The BiLSTM/CRF tail (small FLOPs, serial scans) runs on host, as does
the embedding gather.
"""
import os
import sys
import types
import numpy as np
import ml_dtypes
from scipy.special import erf

V, H, NL, NH, S, B, HL, T = 30522, 768, 12, 12, 256, 32, 256, 9
DH = H // NH
FF = 4 * H
NCORES = 8
BL = B // NCORES          # sentences per core
TOK = BL * S              # tokens per core (1024)
KT = H // 128             # 6 k-tiles over hidden
USE_DEVICE = os.environ.get("KERNEL_HOST", "") == ""
DEV_LAYERS = int(os.environ.get("KERNEL_LAYERS", str(NL)))
TRACE = os.environ.get("KERNEL_TRACE", "") == "1"

LAST_HW_NS = None
_CACHE = {}
BF16 = ml_dtypes.bfloat16


def _install_ntff_shim():
    """The agent image's antenv lacks axon_hooks, which bass_utils
    imports for trace=True under axon. Shim it and install the ctypes
    NTFF hook so exec_time_ns can be measured."""
    if "antenv.axon_hooks" in sys.modules:
        return
    try:
        mod = types.ModuleType("antenv.axon_hooks")
        mod._hook = None

        def set_axon_ntff_profile_hook(h):
            mod._hook = h

        def get_axon_ntff_profile_hook():
            return mod._hook

        mod.set_axon_ntff_profile_hook = set_axon_ntff_profile_hook
        mod.get_axon_ntff_profile_hook = get_axon_ntff_profile_hook
        sys.modules["antenv.axon_hooks"] = mod
        import antenv
        antenv.axon_hooks = mod
        from trn_agent_boot.trn_boot import _ntff_profile_via_ctypes
        mod.set_axon_ntff_profile_hook(
            _ntff_profile_via_ctypes('/opt/axon/libaxon_pjrt.so'))
    except Exception:
        pass


# ---------------------------------------------------------------- host math
def _ln_np(x, g, b):
    m = x.mean(-1, keepdims=True)
    v = ((x - m) ** 2).mean(-1, keepdims=True)
    return (x - m) / np.sqrt(v + 1e-12) * g + b


SIMACT = os.environ.get("KERNEL_SIMACT", "") == "1"   # CoreSim lacks Gelu


def _gelu_np(x):
    if SIMACT:
        return np.tanh(x).astype(np.float32)
    return (0.5 * x * (1.0 + erf(x / np.float32(np.sqrt(2.0))))).astype(np.float32)


def _sigmoid_np(x):
    return 1.0 / (1.0 + np.exp(-x))


def _bert_host(h, a, n_layers=NL):
    Bc = h.shape[0]
    for l in range(n_layers):
        qkv = h @ a['Wqkv'][l] + a['bqkv'][l]
        q, k, v = [t.reshape(Bc, S, NH, DH) for t in np.split(qkv, 3, axis=-1)]
        sc = np.einsum('bqhd,bkhd->bhqk', q, k) / np.float32(np.sqrt(DH))
        sc = sc - sc.max(-1, keepdims=True)
        p = np.exp(sc)
        p = p / p.sum(-1, keepdims=True)
        ctx = np.einsum('bhqk,bkhd->bqhd', p, v).reshape(Bc, S, H)
        h = _ln_np(h + ctx @ a['Wo'][l] + a['bo'][l], a['ln1_g'][l], a['ln1_b'][l])
        ff = _gelu_np(h @ a['W1'][l] + a['b1'][l]) @ a['W2'][l] + a['b2'][l]
        h = _ln_np(h + ff, a['ln2_g'][l], a['ln2_b'][l])
    return h


def _lstm_host(x, Wih, Whh, bih, bhh, reverse):
    Bc = x.shape[0]
    pre = np.swapaxes(x, 0, 1) @ Wih.T + (bih + bhh)  # [S,B,4H]
    hs = np.zeros((S, Bc, HL), np.float32)
    h = np.zeros((Bc, HL), np.float32)
    c = np.zeros((Bc, HL), np.float32)
    order = range(S - 1, -1, -1) if reverse else range(S)
    WhhT = np.ascontiguousarray(Whh.T)
    for t in order:
        g = pre[t] + h @ WhhT
        i, f, gg, o = np.split(g, 4, axis=-1)
        c = _sigmoid_np(f) * c + _sigmoid_np(i) * np.tanh(gg)
        h = _sigmoid_np(o) * np.tanh(c)
        hs[t] = h
    return np.swapaxes(hs, 0, 1)


def _logsumexp(a, axis):
    m = a.max(axis=axis, keepdims=True)
    return (np.log(np.exp(a - m).sum(axis=axis, keepdims=True)) + m).squeeze(axis)


def _crf_host(logits, labels, maskf, crf_start, crf_end, crf_trans):
    em = np.take_along_axis(logits, labels[..., None], -1)[..., 0]
    tr = crf_trans[labels[:, :-1], labels[:, 1:]]
    last_idx = maskf.sum(1).astype(np.int32) - 1
    last_tag = np.take_along_axis(labels, last_idx[:, None], 1)[:, 0]
    num = (crf_start[labels[:, 0]] + em[:, 0]
           + ((em[:, 1:] + tr) * maskf[:, 1:]).sum(1) + crf_end[last_tag])
    alpha = crf_start + logits[:, 0]
    for t in range(1, S):
        nxt = _logsumexp(alpha[:, :, None] + crf_trans[None] + logits[:, t][:, None, :], 1)
        alpha = np.where(maskf[:, t][:, None] > 0, nxt, alpha)
    den = _logsumexp(alpha + crf_end, -1)
    return den - num


# ------------------------------------------------------------ device program
class Prog:
    """Raw-Bass multi-engine program recorder with interval-based
    dependency tracking. Each op declares the [p0,p1)x[c0,c1) boxes it
    reads and writes per buffer; waits are emitted only for overlapping
    RAW / WAR / WAW hazards, deduplicated per consumer engine. Same-
    engine ordering is implicit for compute engines (in-order queues);
    DMA completions are asynchronous so DMA-DMA hazards still wait."""

    ENGINES = ("pe", "act", "dve", "dma")

    def __init__(self):
        self.ops = {e: [] for e in self.ENGINES}
        self.counts = {}
        self.seen = {e: {} for e in self.ENGINES}
        self.sem_names = {"pe", "act", "dve"}
        self.writers = {}   # buf -> [(p0,p1,c0,c1,sem,val,engine)]
        self.readers = {}   # buf -> [(p0,p1,c0,c1,sem,val,engine)]
        self.nwaits = 0

    @staticmethod
    def _ov(b1, b2):
        return b1[0] < b2[1] and b2[0] < b1[1] and b1[2] < b2[3] and b2[2] < b1[3]

    @staticmethod
    def _covers(b1, b2):
        return (b1[0] <= b2[0] and b1[1] >= b2[1]
                and b1[2] <= b2[2] and b1[3] >= b2[3])

    def emit(self, engine, fn, reads=(), writes=(), dma_sem="dmaS0"):
        sem_self = dma_sem if engine == "dma" else engine
        inc = 16 if engine == "dma" else 1
        deps = {}

        def add_dep(sem, val, dep_eng):
            if dep_eng == engine and engine != "dma":
                return                      # in-order compute queue
            if val > deps.get(sem, 0):
                deps[sem] = val

        if engine == "dma":
            self.sem_names.add(sem_self)
            # DMA completions are unordered across in-flight transfers.
            # Keep at most ONE in flight per semaphore: the SP waits for
            # the previous transfer on this sem before posting, so a
            # consumer waiting an intermediate value is sound.
            prev = self.counts.get(sem_self, 0)
            if prev > 0:
                add_dep(sem_self, prev, "dma-prev")

        for box in reads:
            for w in self.writers.get(box[0], ()):
                if self._ov(box[1:], w[:4]):
                    add_dep(w[4], w[5], w[6])
        for box in writes:
            for w in self.writers.get(box[0], ()):
                if self._ov(box[1:], w[:4]):
                    add_dep(w[4], w[5], w[6])
            for r in self.readers.get(box[0], ()):
                if self._ov(box[1:], r[:4]):
                    add_dep(r[4], r[5], r[6])

        waits = []
        for sem, val in deps.items():
            if self.seen[engine].get(sem, 0) < val:
                waits.append((sem, val))
                self.seen[engine][sem] = val
        self.nwaits += len(waits)

        self.counts[sem_self] = self.counts.get(sem_self, 0) + inc
        val_self = self.counts[sem_self]
        for box in writes:
            lst = self.writers.setdefault(box[0], [])
            lst[:] = [w for w in lst if not self._covers(box[1:], w[:4])]
            lst.append((*box[1:], sem_self, val_self, engine))
            # a write also invalidates reader entries it covers from the
            # same engine+older ops is unsafe to drop; keep readers,
            # prune only exact-duplicate boxes from this engine
        for box in reads:
            lst = self.readers.setdefault(box[0], [])
            lst[:] = [r for r in lst
                      if not (r[6] == engine and self._covers(box[1:], r[:4]))]
            lst.append((*box[1:], sem_self, val_self, engine))
        self.ops[engine].append((waits, fn, sem_self, inc))

    def replay(self, engine, eng, sems):
        for waits, fn, sem_self, inc in self.ops[engine]:
            for name, val in waits:
                eng.wait_ge(sems[name], val)
            fn().then_inc(sems[sem_self], inc)


def _build_encoder(n_layers):
    import concourse.bass as bass
    import concourse.mybir as mybir
    from contextlib import ExitStack
    dt = mybir.dt
    f32 = dt.float32
    bf16 = dt.bfloat16
    AF = mybir.ActivationFunctionType
    ALU = mybir.AluOpType
    AF_GELU = AF.Tanh if SIMACT else AF.Gelu

    nc = bass.Bass()
    ctx = ExitStack()

    # ---- DRAM parameters (weights in bf16, biases/stats in f32)
    hT0 = nc.declare_dram_parameter("hT0", [H, TOK], bf16, isOutput=False)
    Wqkv = nc.declare_dram_parameter("Wqkv", [NL, H, 3 * H], bf16, isOutput=False)
    Wo = nc.declare_dram_parameter("Wo", [NL, H, H], bf16, isOutput=False)
    W1 = nc.declare_dram_parameter("W1", [NL, H, FF], bf16, isOutput=False)
    W2 = nc.declare_dram_parameter("W2", [NL, FF, H], bf16, isOutput=False)
    biasall = nc.declare_dram_parameter("biasall", [NL, 128, 80], f32, isOutput=False)
    consts = nc.declare_dram_parameter("consts", [128, 1024], bf16, isOutput=False)
    hTout = nc.declare_dram_parameter("hTout", [H, TOK], bf16, isOutput=True)
    zscr = nc.dram_tensor("zscr", [4, 3072], f32)

    # ---- on-chip tensors
    def sbt(nm, shape, dtype=bf16):
        return ctx.enter_context(nc.sbuf_tensor(nm, shape, dtype))

    hT = sbt("hT", [128, KT, TOK])
    h1T = sbt("h1T", [128, KT, TOK])
    ctxT = sbt("ctxT", [128, KT * TOK])   # flat; viewed [128, 6, 1024]
    big = sbt("bigb", [128, 12, TOK])     # qkT in attn; ff1 0-11; LN sq
    vbuf = sbt("vbuf", [128, KT * TOK])   # v=[128tok,768f]x8 / ff1 18-23
    wsl = sbt("wsl", [128, 3, 3072])      # weight slab, 3 slots
    vwsl = sbt("vwsl", [128, KT, H])      # Wv slab
    bias = sbt("biassb", [128, 2, 80], f32)   # double-buffered per layer
    csts = sbt("csts", [128, 1024])
    stats = sbt("stats", [1, 2048], f32)  # mean | E2/var/istd
    stats2 = sbt("stats2", [1, 1024], f32)
    statsbf = sbt("statsbf", [1, 2048])   # bf16 mean | istd for broadcast
    zbuf4 = sbt("zbuf4", [97, 3072], f32)  # Z at partition bases 0/32/64/96
    zbuf = sbt("zbuf", [12, 1024], f32)   # Z regrouped per head
    zbufb = sbt("zbufb", [12, 1024])      # bf16 1/Z
    lnb = sbt("lnb", [128, 2, 1024])      # bf16 bcast mean | istd
    zsb = sbt("zsb", [128, KT, 1024])     # bf16 bcast 1/Z per m-tile
    expS = sbt("expS", [128, 3, 2, S])    # parity x k-half x q

    psA = ctx.enter_context(nc.psum_tensor("psA", [128, 1024], f32))
    psB = ctx.enter_context(nc.psum_tensor("psB", [128, 1024], f32))
    pS0 = ctx.enter_context(nc.psum_tensor("pS0", [128, 2, S], f32))
    pS1 = ctx.enter_context(nc.psum_tensor("pS1", [128, 2, S], f32))
    pC0 = ctx.enter_context(nc.psum_tensor("pC0", [128, S], f32))
    pC1 = ctx.enter_context(nc.psum_tensor("pC1", [128, S], f32))

    ctxTv = ctxT[:, :].rearrange("p (n t) -> p n t", t=TOK)

    P = Prog()

    def dma(dst, src, reads=(), writes=(), sem="dmaB"):
        P.emit("dma", lambda d=dst, s=src: nc.sync.dma_start(out=d, in_=s),
               reads=reads, writes=writes, dma_sem=sem)

    def mm(out, lhsT, rhs, start, stop, reads=(), writes=()):
        P.emit("pe", lambda o=out, l=lhsT, r=rhs, a=start, b=stop:
               nc.tensor.matmul(o, l, r, start=a, stop=b),
               reads=reads, writes=writes)

    def act(out, in_, func, b=0.0, scale=1.0, reads=(), writes=()):
        P.emit("act", lambda o=out, i=in_, f=func, bb=b, s=scale:
               nc.scalar.activation(o, i, f, bias=bb, scale=s),
               reads=reads, writes=writes)

    def dve_tt(out, in0, in1, op, reads=(), writes=()):
        P.emit("dve", lambda o=out, x=in0, y=in1, z=op:
               nc.vector.tensor_tensor(o, x, y, z), reads=reads, writes=writes)

    def dve_ts(out, in_, s1, s2, op0, op1, reads=(), writes=()):
        P.emit("dve", lambda o=out, i=in_, a=s1, b=s2, x=op0, y=op1:
               nc.vector.tensor_scalar(o, i, a, b, x, y),
               reads=reads, writes=writes)

    # ---- box helpers: (buf, p0, p1, c0, c1) in each buffer's flat cols
    def bx(name, p0, p1, c0, c1):
        return (name, p0, p1, c0, c1)

    def hT_b(kt, c0=0, c1=TOK):
        return bx("hT", 0, 128, kt * TOK + c0, kt * TOK + c1)

    def h1T_b(kt, c0=0, c1=TOK):
        return bx("h1T", 0, 128, kt * TOK + c0, kt * TOK + c1)

    def ctx_b(n, c0=0, c1=TOK, p0=0, p1=128):
        return bx("ctxT", p0, p1, n * TOK + c0, n * TOK + c1)

    def big_b(n, c0=0, c1=TOK, p0=0, p1=128):
        return bx("big", p0, p1, n * TOK + c0, n * TOK + c1)

    def v_b(c0, c1, p0=0, p1=128):
        return bx("vbuf", p0, p1, c0, c1)

    def wsl_b(slot, c0=0, c1=3072):
        return bx("wsl", 0, 128, slot * 3072 + c0, slot * 3072 + c1)

    def psA_b(c0=0, c1=1024, p0=0, p1=128):
        return bx("psA", p0, p1, c0, c1)

    def psB_b(c0=0, c1=1024, p0=0, p1=128):
        return bx("psB", p0, p1, c0, c1)

    # ---- boot: constants + initial activations
    dma(csts[:, :], consts[:, :], writes=(bx("csts", 0, 128, 0, 1024),))
    dma(hT[:, :, :], hT0.rearrange("(n p) t -> p n t", p=128),
        writes=(bx("hT", 0, 128, 0, KT * TOK),))
    ones128 = csts[:, 0:1]            # bf16 ones column
    onesrow = csts[0:1, 2:130]        # bf16 [1,128] ones on partition 0
    CSTS_R = (bx("csts", 0, 128, 0, 1024),)

    def m12(m):                       # [12, 128] head-broadcast map k-tile
        return csts[0:12, 130 + m * 128:130 + (m + 1) * 128]

    def stream_gemm(W_slab_fn, n_in, n_out, rhs_tile_fn, rhs_box_fn,
                    out_fn, out_box_fn, bias_col_fn, act_fn, lparity):
        """out[m] = act(sum_kt W[kt,m].T @ rhs[kt] + bias[m]); one slab
        DMA per m-tile through 3 wsl slots; psA/psB alternate per m."""
        kt_n = n_in // 128
        mt_n = n_out // 128
        for m in range(mt_n):
            slot = m % 3
            wdst = wsl[:, slot, 0:kt_n * 128].rearrange(
                "p (n m) -> p n m", m=128)
            dma(wdst, W_slab_fn(m), writes=(wsl_b(slot, 0, kt_n * 128),),
                sem="dmaW%d" % slot)
            ps, ps_box = (psA, psA_b) if m % 2 == 0 else (psB, psB_b)
            for half in range(2):
                c0, c1 = half * 512, (half + 1) * 512
                for kt in range(kt_n):
                    mm(ps[:, c0:c1],
                       wsl[:, slot, kt * 128:(kt + 1) * 128],
                       rhs_tile_fn(kt)[:, c0:c1],
                       start=(kt == 0), stop=(kt == kt_n - 1),
                       reads=(wsl_b(slot, kt * 128, (kt + 1) * 128),
                              rhs_box_fn(kt, c0, c1)),
                       writes=(ps_box(c0, c1),))
            act(out_fn(m), ps[:, 0:TOK], act_fn,
                b=bias[:, lparity, bias_col_fn(m):bias_col_fn(m) + 1],
                reads=(ps_box(0, TOK),
                       bx("bias", 0, 128, lparity * 80 + bias_col_fn(m),
                          lparity * 80 + bias_col_fn(m) + 1)),
                writes=(out_box_fn(m),))

    def layernorm(x, xb, gcol0, bcol0, lparity):
        """In-place LN over features of x=[128,KT,TOK] (bf16), fp32
        stats. Uses big[:,0:KT] as square scratch."""
        # x row-sums first (no dependency on squares)
        for half in range(2):
            c0, c1 = half * 512, (half + 1) * 512
            for kt in range(KT):
                mm(psA[0:1, c0:c1], ones128, x[:, kt, c0:c1],
                   start=(kt == 0), stop=(kt == KT - 1),
                   reads=CSTS_R + (xb(kt, c0, c1),),
                   writes=(psA_b(c0, c1, 0, 1),))
        for kt in range(KT):
            act(big[:, kt, :], x[:, kt, :], AF.Square,
                reads=(xb(kt),), writes=(big_b(kt),))
        for half in range(2):
            c0, c1 = half * 512, (half + 1) * 512
            for kt in range(KT):
                mm(psB[0:1, c0:c1], ones128, big[:, kt, c0:c1],
                   start=(kt == 0), stop=(kt == KT - 1),
                   reads=CSTS_R + (big_b(kt, c0, c1),),
                   writes=(psB_b(c0, c1, 0, 1),))
        mean = stats[0:1, 0:1024]
        blk = stats[0:1, 1024:2048]       # E2 -> var -> istd, in place
        tmp = stats2[0:1, :]              # meansq -> sd
        MEAN_B = (bx("stats", 0, 1, 0, 1024),)
        BLK_B = (bx("stats", 0, 1, 1024, 2048),)
        TMP_B = (bx("stats2", 0, 1, 0, 1024),)
        act(mean, psA[0:1, :], AF.Identity, scale=1.0 / H,
            reads=(psA_b(0, 1024, 0, 1),), writes=MEAN_B)
        act(blk, psB[0:1, :], AF.Identity, scale=1.0 / H,
            reads=(psB_b(0, 1024, 0, 1),), writes=BLK_B)
        dve_tt(tmp, mean, mean, ALU.mult, reads=MEAN_B, writes=TMP_B)
        dve_tt(blk, blk, tmp, ALU.subtract, reads=BLK_B + TMP_B, writes=BLK_B)
        # istd = exp(-0.5*ln(var+eps)); AF.Rsqrt is rejected by bass and
        # a [1,1024] DVE reciprocal costs ~8us (column-serial)
        P.emit("dve", lambda: nc.vector.tensor_scalar_add(blk, blk, 1e-12),
               reads=BLK_B, writes=BLK_B)
        act(tmp, blk, AF.Ln, reads=BLK_B, writes=TMP_B)
        act(tmp, tmp, AF.Exp, scale=-0.5, reads=TMP_B, writes=TMP_B)
        # bf16 copies for the broadcast matmuls
        MEANB_B = (bx("statsbf", 0, 1, 0, 1024),)
        ISTDB_B = (bx("statsbf", 0, 1, 1024, 2048),)
        act(statsbf[0:1, 0:1024], mean, AF.Identity,
            reads=MEAN_B, writes=MEANB_B)
        act(statsbf[0:1, 1024:2048], tmp, AF.Identity,
            reads=TMP_B, writes=ISTDB_B)
        for half in range(2):
            c0, c1 = half * 512, (half + 1) * 512
            mm(psA[:, c0:c1], onesrow, statsbf[0:1, c0:c1], start=True,
               stop=True, reads=CSTS_R + MEANB_B, writes=(psA_b(c0, c1),))
            mm(psB[:, c0:c1], onesrow, statsbf[0:1, 1024 + c0:1024 + c1],
               start=True, stop=True, reads=CSTS_R + ISTDB_B,
               writes=(psB_b(c0, c1),))
        # copy broadcasts to SBUF bf16: frees PSUM for the next GEMM
        # stream immediately and doubles DVE rate in the apply passes
        act(lnb[:, 0, :], psA[:, 0:TOK], AF.Identity,
            reads=(psA_b(0, TOK),), writes=(bx("lnb", 0, 128, 0, 1024),))
        act(lnb[:, 1, :], psB[:, 0:TOK], AF.Identity,
            reads=(psB_b(0, TOK),), writes=(bx("lnb", 0, 128, 1024, 2048),))
        for kt in range(KT):
            dve_tt(x[:, kt, :], x[:, kt, :], lnb[:, 0, :], ALU.subtract,
                   reads=(xb(kt), bx("lnb", 0, 128, 0, 1024)),
                   writes=(xb(kt),))
            dve_tt(x[:, kt, :], x[:, kt, :], lnb[:, 1, :], ALU.mult,
                   reads=(xb(kt), bx("lnb", 0, 128, 1024, 2048)),
                   writes=(xb(kt),))
            dve_ts(x[:, kt, :], x[:, kt, :],
                   bias[:, lparity, gcol0 + kt:gcol0 + kt + 1],
                   bias[:, lparity, bcol0 + kt:bcol0 + kt + 1],
                   ALU.mult, ALU.add,
                   reads=(xb(kt),
                          bx("bias", 0, 128, lparity * 80 + gcol0 + kt,
                             lparity * 80 + gcol0 + kt + 1),
                          bx("bias", 0, 128, lparity * 80 + bcol0 + kt,
                             lparity * 80 + bcol0 + kt + 1)),
                   writes=(xb(kt),))

    def vtile(m):                     # v token-tile m: [128, 768]
        return vbuf[:, m * H:(m + 1) * H]

    def fftile(kt):                   # ff1 feature k-tile: [128, 1024]
        if kt < 12:
            return big[:, kt, :]
        if kt < 18:
            return ctxTv[:, kt - 12, :]
        return vbuf[:, (kt - 18) * TOK:(kt - 17) * TOK]

    def fftile_b(kt, c0=0, c1=TOK):
        if kt < 12:
            return big_b(kt, c0, c1)
        if kt < 18:
            return ctx_b(kt - 12, c0, c1)
        return v_b((kt - 18) * TOK + c0, (kt - 18) * TOK + c1)

    for l in range(n_layers):
        lp = l % 2
        dma(bias[:, lp, :], biasall[l],
            writes=(bx("bias", 0, 128, lp * 80, lp * 80 + 80),))

        # qkT into big[:, 0:12]: features q(0-5) k(6-11)
        Wq_r = Wqkv[l].rearrange("(n p) m -> p n m", p=128)
        stream_gemm(lambda m: Wq_r[:, :, m * 128:(m + 1) * 128], H, 1536,
                    lambda kt: hT[:, kt, :], hT_b,
                    lambda m: big[:, m, :], big_b,
                    lambda m: m, AF.Identity, lp)

        # v = hT.T @ Wv  (token-major; bias folded in after softmax)
        dma(vwsl[:, :, :], Wq_r[:, :, 1536:2304],
            writes=(bx("vwsl", 0, 128, 0, KT * H),), sem="dmaV")
        for m in range(8):
            ps, ps_box = (psA, psA_b) if m % 2 == 0 else (psB, psB_b)
            for c0, c1 in ((0, 512), (512, 768)):
                for kt in range(KT):
                    mm(ps[:, c0:c1], hT[:, kt, m * 128:(m + 1) * 128],
                       vwsl[:, kt, c0:c1],
                       start=(kt == 0), stop=(kt == KT - 1),
                       reads=(hT_b(kt, m * 128, (m + 1) * 128),
                              bx("vwsl", 0, 128, kt * H + c0, kt * H + c1)),
                       writes=(ps_box(c0, c1),))
            act(vtile(m), ps[:, 0:H], AF.Identity,
                reads=(ps_box(0, H),), writes=(v_b(m * H, (m + 1) * H),))

        # attention: software-pipelined 2 heads ahead; 3-way score parity
        # (pS0, pS1, and psA[:,0:512] which is idle during the head loop)
        def ps_sc(t):
            par = t % 3
            if par == 0:
                return (lambda i: pS0[:, i, :]), "pS0", pS0[:, :, :], None
            if par == 1:
                return (lambda i: pS1[:, i, :]), "pS1", pS1[:, :, :], None
            return ((lambda i: psA[:, i * S:(i + 1) * S]), "psA",
                    psA[:, 0:2 * S].rearrange("p (i s) -> p i s", s=S), None)

        def zs_ap(t):
            par = t % 3
            if par == 0:
                return pS0[0:1, 0, 0:S]
            if par == 1:
                return pS1[0:1, 0, 0:S]
            return psA[0:1, 0:S]

        def head_sc(t):
            s, hh = divmod(t, NH)
            par = t % 3
            pS_i, pSn, pS_full, _ = ps_sc(t)
            prow = 64 * (hh % 2)
            qt = big[prow:prow + 64, hh // 2, s * S:(s + 1) * S]
            for i in range(2):
                ktap = big[prow:prow + 64,
                           6 + hh // 2, s * S + i * 128:s * S + (i + 1) * 128]
                mm(pS_i(i), ktap, qt, start=True, stop=True,
                   reads=(big_b(6 + hh // 2, s * S + i * 128,
                                s * S + (i + 1) * 128, prow, prow + 64),
                          big_b(hh // 2, s * S, (s + 1) * S, prow, prow + 64)),
                   writes=(bx(pSn, 0, 128, i * S, (i + 1) * S),))
            act(expS[:, par, :, :], pS_full, AF.Exp, scale=1.0 / 8.0,
                reads=(bx(pSn, 0, 128, 0, 2 * S),),
                writes=(bx("expS", 0, 128, par * 2 * S, (par + 1) * 2 * S),))

        def head_pv(t):
            s, hh = divmod(t, NH)
            par = t % 3
            pS_i, pSn, _, _ = ps_sc(t)
            EX_R = (bx("expS", 0, 128, par * 2 * S, (par + 1) * 2 * S),)
            cpar = t % 2
            pC = pC0 if cpar == 0 else pC1
            pCn = "pC0" if cpar == 0 else "pC1"
            for i in range(2):
                mm(pC[0:64, :],
                   vtile(2 * s + i)[:, hh * 64:(hh + 1) * 64],
                   expS[:, par, i, :], start=(i == 0), stop=(i == 1),
                   reads=(v_b((2 * s + i) * H + hh * 64,
                              (2 * s + i) * H + (hh + 1) * 64),) + EX_R,
                   writes=(bx(pCn, 0, 64, 0, S),))
                mm(zs_ap(t), ones128, expS[:, par, i, :],
                   start=(i == 0), stop=(i == 1),
                   reads=CSTS_R + EX_R,
                   writes=(bx(pSn, 0, 1, 0, S),))
            prow = 64 * (hh % 2)
            act(ctxTv[prow:prow + 64, hh // 2, s * S:(s + 1) * S],
                pC[0:64, :], AF.Identity,
                reads=(bx(pCn, 0, 64, 0, S),),
                writes=(ctx_b(hh // 2, s * S, (s + 1) * S, prow, prow + 64),))
            zr = zbuf4[32 * (hh % 4):32 * (hh % 4) + 1,
                       (hh // 4) * 1024 + s * S:(hh // 4) * 1024 + (s + 1) * S]
            act(zr, zs_ap(t), AF.Identity,
                reads=(bx(pSn, 0, 1, 0, S),),
                writes=(bx("zbuf4", 32 * (hh % 4), 32 * (hh % 4) + 1,
                           (hh // 4) * 1024 + s * S,
                           (hh // 4) * 1024 + (s + 1) * S),))

        head_sc(0)
        head_sc(1)
        for t in range(2, BL * NH):
            head_sc(t)
            head_pv(t - 2)
        head_pv(BL * NH - 2)
        head_pv(BL * NH - 1)

        # normalize ctx by Z (per head), add v bias. Z sums land in zbuf4
        # rows (partition bases 0/32/64/96); one SBUF->SBUF DMA regroups
        # them to [12, 1024], then a single 12-partition-parallel
        # reciprocal emits bf16 1/Z directly.
        dma(zscr[:, :], zbuf4[0:97:32, :],
            reads=(bx("zbuf4", 0, 97, 0, 3072),),
            writes=(bx("zscr", 0, 4, 0, 3072),), sem="dmaZ")
        dma(zbuf[0:12, :], zscr[:, :].rearrange("p (b t) -> (p b) t", b=3),
            reads=(bx("zscr", 0, 4, 0, 3072),),
            writes=(bx("zbuf", 0, 12, 0, 1024),), sem="dmaZ")
        def _recip_z():
            with nc.allow_low_precision(reason="1/Z feeds bf16 matmul"):
                return nc.vector.reciprocal(zbufb[0:12, :], zbuf[0:12, :])
        P.emit("dve", _recip_z,
               reads=(bx("zbuf", 0, 12, 0, 1024),),
               writes=(bx("zbufb", 0, 12, 0, 1024),))
        for m in range(KT):
            ps, ps_box = (psA, psA_b) if m % 2 == 0 else (psB, psB_b)
            for half in range(2):
                c0, c1 = half * 512, (half + 1) * 512
                mm(ps[:, c0:c1], m12(m), zbufb[0:12, c0:c1], start=True,
                   stop=True,
                   reads=CSTS_R + (bx("zbufb", 0, 12, c0, c1),),
                   writes=(ps_box(c0, c1),))
            act(zsb[:, m, :], ps[:, 0:TOK], AF.Identity,
                reads=(ps_box(0, TOK),),
                writes=(bx("zsb", 0, 128, m * 1024, (m + 1) * 1024),))
            dve_tt(ctxTv[:, m, :], ctxTv[:, m, :], zsb[:, m, :], ALU.mult,
                   reads=(ctx_b(m), bx("zsb", 0, 128, m * 1024, (m + 1) * 1024)),
                   writes=(ctx_b(m),))
            P.emit("dve", lambda m=m, lp=lp: nc.vector.tensor_scalar_add(
                ctxTv[:, m, :], ctxTv[:, m, :], bias[:, lp, 12 + m:13 + m]),
                reads=(ctx_b(m), bx("bias", 0, 128, lp * 80 + 12 + m,
                                    lp * 80 + 13 + m)),
                writes=(ctx_b(m),))

        # attn proj + residual + LN1
        Wo_r = Wo[l].rearrange("(n p) m -> p n m", p=128)
        stream_gemm(lambda m: Wo_r[:, :, m * 128:(m + 1) * 128], H, H,
                    lambda kt: ctxTv[:, kt, :], ctx_b,
                    lambda m: h1T[:, m, :], h1T_b,
                    lambda m: 18 + m, AF.Identity, lp)
        for m in range(KT):
            dve_tt(h1T[:, m, :], h1T[:, m, :], hT[:, m, :], ALU.add,
                   reads=(h1T_b(m), hT_b(m)), writes=(h1T_b(m),))
        layernorm(h1T, h1T_b, 24, 30, lp)

        # FF1 (gelu) into big/ctxT/vbuf tiles
        W1_r = W1[l].rearrange("(n p) m -> p n m", p=128)
        stream_gemm(lambda m: W1_r[:, :, m * 128:(m + 1) * 128], H, FF,
                    lambda kt: h1T[:, kt, :], h1T_b,
                    fftile, fftile_b,
                    lambda m: 36 + m, AF_GELU, lp)

        # FF2 + residual + LN2 -> hT
        W2_r = W2[l].rearrange("(n p) m -> p n m", p=128)
        stream_gemm(lambda m: W2_r[:, :, m * 128:(m + 1) * 128], FF, H,
                    fftile, fftile_b,
                    lambda m: hT[:, m, :], hT_b,
                    lambda m: 60 + m, AF.Identity, lp)
        for m in range(KT):
            dve_tt(hT[:, m, :], hT[:, m, :], h1T[:, m, :], ALU.add,
                   reads=(hT_b(m), h1T_b(m)), writes=(hT_b(m),))
        layernorm(hT, hT_b, 66, 72, lp)

    dma(hTout.rearrange("(n p) t -> p n t", p=128), hT[:, :, :],
        reads=(bx("hT", 0, 128, 0, KT * TOK),),
        writes=(bx("hTout", 0, 128, 0, KT * TOK),))

    # ---- replay into engine blocks
    sems = {}
    for name in sorted(P.sem_names):
        sems[name] = ctx.enter_context(nc.semaphore(name))
    with nc.Block() as block:
        @block.tensor
        def _(eng):
            P.replay("pe", eng, sems)

        @block.scalar
        def _(eng):
            P.replay("act", eng, sems)

        @block.vector
        def _(eng):
            P.replay("dve", eng, sems)

        @block.sync
        def _(eng):
            P.replay("dma", eng, sems)

    return nc, ctx


def _pack_consts():
    c = np.zeros((128, 1024), np.float32)
    c[:, 0] = 1.0                       # ones128
    c[0, 2:130] = 1.0                   # onesrow
    # zbuf row r (after the strided reshape DMA) holds head (r%3)*4 + r//3
    for r in range(NH):
        hh = (r % 3) * 4 + r // 3
        for f in range(H):
            if f // DH == hh:
                c[r, 130 + f] = 1.0
    return c.astype(BF16)


def _pack_bias(a):
    out = np.zeros((NL, 128, 80), np.float32)

    def col(vec):                       # feature vec [n*128] -> [128, n]
        return vec.reshape(-1, 128).T

    for l in range(NL):
        out[l, :, 0:18] = col(a['bqkv'][l])
        out[l, :, 18:24] = col(a['bo'][l])
        out[l, :, 24:30] = col(a['ln1_g'][l])
        out[l, :, 30:36] = col(a['ln1_b'][l])
        out[l, :, 36:60] = col(a['b1'][l])
        out[l, :, 60:66] = col(a['b2'][l])
        out[l, :, 66:72] = col(a['ln2_g'][l])
        out[l, :, 72:78] = col(a['ln2_b'][l])
    return out


def run_device(h0, a):
    global LAST_HW_NS
    if not USE_DEVICE:
        return _bert_host(h0, a)
    import time
    if TRACE:
        _install_ntff_shim()
    from concourse.bass_utils import run_bass_kernel_spmd

    key = ("enc", DEV_LAYERS)
    if key not in _CACHE:
        _CACHE[key] = _build_encoder(DEV_LAYERS)
    nc, _ctx = _CACHE[key]

    biasall = _pack_bias(a)
    consts = _pack_consts()
    shared = {"Wqkv": a['Wqkv'].astype(BF16), "Wo": a['Wo'].astype(BF16),
              "W1": a['W1'].astype(BF16), "W2": a['W2'].astype(BF16),
              "biasall": biasall, "consts": consts}
    in_maps = []
    for c in range(NCORES):
        hc = h0[c * BL:(c + 1) * BL].reshape(TOK, H).T
        in_maps.append(dict(shared, hT0=np.ascontiguousarray(hc).astype(BF16)))

    t0 = time.time()
    res = run_bass_kernel_spmd(nc, in_maps, list(range(NCORES)), trace=TRACE)
    wall_ns = int((time.time() - t0) * 1e9)
    LAST_HW_NS = res.exec_time_ns if res.exec_time_ns else wall_ns

    h = np.zeros((B, S, H), np.float32)
    for c in range(NCORES):
        h[c * BL:(c + 1) * BL] = \
            res.results[c]["hTout"].astype(np.float32).T.reshape(BL, S, H)
    if DEV_LAYERS < NL:                 # debugging path: finish on host
        a2 = {k: (v[DEV_LAYERS:] if k in ("Wqkv", "bqkv", "Wo", "bo", "ln1_g",
              "ln1_b", "W1", "b1", "W2", "b2", "ln2_g", "ln2_b") else v)
              for k, v in a.items()}
        h = _bert_host(h, a2, NL - DEV_LAYERS)
    return h


def kernel(input_ids, attention_mask, labels, emb_tok, emb_pos, emb_type,
           ln_emb_g, ln_emb_b, Wqkv, bqkv, Wo, bo, ln1_g, ln1_b, W1, b1,
           W2, b2, ln2_g, ln2_b, Wih_f, Whh_f, bih_f, bhh_f, Wih_b, Whh_b,
           bih_b, bhh_b, Wc, bc, tag_weight, crf_start, crf_end, crf_trans):
    args = {k: np.asarray(v) for k, v in locals().items()}
    maskf = args['attention_mask'].astype(np.float32)

    h0 = (args['emb_tok'][args['input_ids']] + args['emb_pos'][:S][None]
          + args['emb_type'][0][None, None]).astype(np.float32)
    h0 = _ln_np(h0, args['ln_emb_g'], args['ln_emb_b'])

    h = run_device(h0, args)

    hf = _lstm_host(h, args['Wih_f'], args['Whh_f'], args['bih_f'], args['bhh_f'], False)
    hb = _lstm_host(h, args['Wih_b'], args['Whh_b'], args['bih_b'], args['bhh_b'], True)
    logits = (np.concatenate([hf, hb], -1) @ args['Wc'] + args['bc']) * args['tag_weight']
    ll = _crf_host(logits, args['labels'], maskf, args['crf_start'],
                   args['crf_end'], args['crf_trans'])
    return np.float32(ll.mean())
